# revision 39
# baseline (speedup 1.0000x reference)
"""Trainium2 Bass kernel for a 16-head causal self-attention block.

Reference computation (B=1, S=4096, H=2048, 16 heads x 128 dim, fp32):
    qkv = x @ w_qkv.T            # [S, 6144]
    q, k = rope(q), rope(k)      # half-split rope
    attn = causal_softmax(q k^T / sqrt(128)) @ v
    out  = attn @ w_o.T          # [S, 2048]

Sharding: tensor-parallel over heads.  Each of the 8 cores owns 2 heads:
it computes its slice of the QKV projection (768 rows), attention for its
2 heads, and a partial o_proj ([S, 2048], bf16); the host sums the 8
partials.

Dataflow per core (matmul operands bf16, accumulation/softmax fp32):
  phase 1 (QKV+rope), per s-tile of 512: one batched x^T DMA group per
    tile, then three matmul sweeps (V, Q, K) rotating over three PSUM
    slots so each sweep's epilogue drains while the next computes.  RoPE
    is fused into the Q/K epilogues; rotate_half is a signed-permutation
    matmul on the PE; cos/sin tables are pre-doubled so one [128,1024]
    DVE op covers both heads.
  phase 2 (attention), per q-tile of 512: loop causal k-chunks of 128:
    scoresT[k,q] -> ACT exp (both heads in one [128,1024] instr) ->
    diagonal-mask multiply -> PV matmul, with bf16 softmax-denominator
    accumulators (two alternating, halves the serial add chain); per
    q-tile a ones-matmul folds the partition sums, fast-reciprocal +
    one multiply normalizes attn^T.  o_proj for q-tile t-1 is
    interleaved into tile t's chunk loop (PE absorbs the exp-paced
    slack); its PSUM drains go to the otherwise-idle GpSimd engine.
"""

import numpy as np

import concourse.bass as bass
import concourse.mybir as mybir
import concourse.tile as tile
from concourse import bacc
from concourse.bass_utils import run_bass_kernel_spmd

F32 = mybir.dt.float32
BF16 = mybir.dt.bfloat16

S = 4096
H = 2048
DH = 128
NH = 16
NCORES = 8
HPC = NH // NCORES          # 2 heads per core
OLOC = HPC * DH             # 256 local o-channels per q/k/v group
P = 128
ST1 = 512                   # phase-1 s-tile width
NHT = H // P                # 16 h-chunks
QT = 512                    # phase-2 q-tile width
NQT = S // QT               # 8 q-tiles
NKC = S // P                # 32 k-chunks
SCALE = 1.0 / float(np.sqrt(np.float32(DH)))

_PROGRAM = None


def _build_body(tc):
    nc = tc.nc

    # x and w_qkv are host-retiled so each partition's chunk is one long
    # contiguous run (big DMA descriptors: ~25ns fixed cost per descriptor)
    xTt = nc.dram_tensor("xTt", [NQT, P, NHT, ST1], BF16, kind="ExternalInput").ap()
    wqt = nc.dram_tensor("wqt", [P, NHT, 3 * OLOC], BF16, kind="ExternalInput").ap()
    woT = nc.dram_tensor("woT", [OLOC, H], BF16, kind="ExternalInput").ap()
    rope = nc.dram_tensor("rope", [P, 2, NQT * 2 * ST1], F32, kind="ExternalInput").ap()
    swapj = nc.dram_tensor("swapj", [P, P], BF16, kind="ExternalInput").ap()
    onesin = nc.dram_tensor("onesin", [P, P], BF16, kind="ExternalInput").ap()
    masks = nc.dram_tensor("masks", [4, P, 2 * QT], BF16, kind="ExternalInput").ap()
    out = nc.dram_tensor("out", [S, H], BF16, kind="ExternalOutput").ap()

    woT_v = woT.rearrange("(t p) h -> p t h", p=P)      # [128, 2, 2048]

    with tc.tile_pool(name="resident", bufs=1) as resident:
        # d-major Q^T/K^T: [128 d, head, s]; s-major V: [128 s, k-chunk, 256]
        QT_sb = resident.tile([P, HPC, S], BF16)
        KT_sb = resident.tile([P, HPC, S], BF16)
        V_sb = resident.tile([P, NKC, OLOC], BF16)
        wT_sb = resident.tile([P, NHT, 3 * OLOC], BF16)
        woT_sb = resident.tile([P, HPC, H], BF16)
        masks_sb = resident.tile([P, 4, 2 * QT], BF16)
        ones_sb = resident.tile([P, P], BF16)
        J_sb = resident.tile([P, P], BF16)

        # All phase-1 inputs go on the sync queue in consumption order so
        # the first-needed transfers get the full DMA bandwidth instead of
        # fair-sharing it with later ones (issue spacing staggers them).
        nc.gpsimd.dma_start(J_sb, swapj)

        # ---------------- phase 1: QKV projection + rope ----------------
        with (
            tc.tile_pool(name="p1x", bufs=2) as p1x,
            tc.tile_pool(name="p1tab", bufs=2) as p1tab,
            tc.tile_pool(name="p1t1", bufs=2) as p1t1,
            tc.tile_pool(name="p1t2", bufs=2) as p1t2,
            tc.tile_pool(name="p1ps", bufs=3, space="PSUM") as p1ps,
            tc.tile_pool(name="p1rot", bufs=1, space="PSUM") as p1rot,
        ):
            def rope_block(blk, dst, s0, cos, sin):
                # blk: [128, 1024] PSUM (both heads); dst: QT_sb or KT_sb
                t1 = p1t1.tile([P, 2 * ST1], F32, tag="t1")
                t2 = p1t2.tile([P, 2 * ST1], BF16, tag="t2")
                nc.vector.tensor_mul(t1, blk, cos)
                nc.vector.tensor_mul(t2, blk, sin)
                rot = p1rot.tile([P, 2 * ST1], F32, tag="rot")
                nc.tensor.matmul(rot[:, 0:ST1], lhsT=J_sb, rhs=t2[:, 0:ST1],
                                 start=True, stop=True)
                nc.tensor.matmul(rot[:, ST1:2 * ST1], lhsT=J_sb,
                                 rhs=t2[:, ST1:2 * ST1], start=True, stop=True)
                for h in range(HPC):
                    nc.vector.tensor_add(
                        dst[:, h, s0:s0 + ST1],
                        t1[:, h * ST1:(h + 1) * ST1],
                        rot[:, h * ST1:(h + 1) * ST1],
                    )

            for st in range(S // ST1):
                s0 = st * ST1
                xt = p1x.tile([P, NHT, ST1], BF16, tag="xt", name=f"xt{st}")
                if st == 0:
                    # fine-grained interleave for the startup-critical tile
                    for g in range(8):
                        nc.sync.dma_start(
                            wT_sb[:, 2 * g:2 * (g + 1), :],
                            wqt[:, 2 * g:2 * (g + 1), :],
                        )
                        nc.sync.dma_start(
                            xt[:, 2 * g:2 * (g + 1), :],
                            xTt[st, :, 2 * g:2 * (g + 1), :],
                        )
                else:
                    for g in range(4):
                        nc.sync.dma_start(
                            xt[:, 4 * g:4 * (g + 1), :],
                            xTt[st, :, 4 * g:4 * (g + 1), :],
                        )
                tab = p1tab.tile([P, 2, 2 * ST1], F32, tag="tab")
                nc.sync.dma_start(
                    tab, rope[:, :, st * 2 * ST1:(st + 1) * 2 * ST1]
                )
                if st == 2:
                    # phase-2-only tensors: issued on the scalar queue here,
                    # behind two tiles of V-drain copies, so their traffic
                    # cannot crowd out the startup-critical loads above
                    nc.scalar.dma_start(ones_sb, onesin)
                    nc.scalar.dma_start(masks_sb, masks.rearrange("j p q -> p j q"))
                    nc.scalar.dma_start(woT_sb, woT_v)
                cos = tab[:, 0, :]
                sin = tab[:, 1, :]

                # --- V sweep ---
                # [128, 1024]: s-subs 0,1 share zero-region/bank 0; 2,3 share 1
                pv = p1ps.tile([P, 2 * ST1], F32, tag="qkv", name=f"pv{st}")
                for ht in range(NHT):
                    for sub in range(ST1 // P):
                        nc.tensor.matmul(
                            pv[:, sub * OLOC:(sub + 1) * OLOC],
                            lhsT=xt[:, ht, sub * P:(sub + 1) * P],
                            rhs=wT_sb[:, ht, 2 * OLOC:3 * OLOC],
                            start=(ht == 0) and sub % 2 == 0,
                            stop=(ht == NHT - 1) and sub % 2 == 1,
                        )
                for sub in range(ST1 // P):
                    nc.scalar.activation(
                        V_sb[:, st * (ST1 // P) + sub, :],
                        pv[:, sub * OLOC:(sub + 1) * OLOC],
                        mybir.ActivationFunctionType.Copy,
                    )

                # --- Q sweep + rope ---
                pq = p1ps.tile([P, 2 * ST1], F32, tag="qkv", name=f"pq{st}")
                for ht in range(NHT):
                    for h in range(HPC):
                        nc.tensor.matmul(
                            pq[:, h * ST1:(h + 1) * ST1],
                            lhsT=wT_sb[:, ht, h * P:(h + 1) * P],
                            rhs=xt[:, ht, :],
                            start=ht == 0, stop=ht == NHT - 1,
                        )
                rope_block(pq, QT_sb, s0, cos, sin)

                # --- K sweep + rope ---
                pk = p1ps.tile([P, 2 * ST1], F32, tag="qkv", name=f"pk{st}")
                for ht in range(NHT):
                    for h in range(HPC):
                        nc.tensor.matmul(
                            pk[:, h * ST1:(h + 1) * ST1],
                            lhsT=wT_sb[:, ht, OLOC + h * P:OLOC + (h + 1) * P],
                            rhs=xt[:, ht, :],
                            start=ht == 0, stop=ht == NHT - 1,
                        )
                rope_block(pk, KT_sb, s0, cos, sin)

        # ---------------- phase 2: attention + interleaved o_proj ---------
        with (
            tc.tile_pool(name="p2e", bufs=4) as p2e,
            tc.tile_pool(name="p2acc", bufs=2) as p2acc,
            tc.tile_pool(name="p2rec", bufs=2) as p2rec,
            tc.tile_pool(name="p2a", bufs=4) as p2a,
            tc.tile_pool(name="p2st", bufs=4) as p2st,
            tc.tile_pool(name="p2sc", bufs=2, space="PSUM") as p2sc,
            tc.tile_pool(name="p2pv", bufs=1, space="PSUM") as p2pv,
            tc.tile_pool(name="p2po", bufs=2, space="PSUM") as p2po,
        ):
            A_tiles = [None] * 4            # A_tiles[t % 4] = attnT of tile t
            stg_cur = [None]
            pending = []                    # (tau, d) o_proj duos not yet issued
            ucount = [0]

            def oproj_duo(tau, d):
                # one of 8 o_proj duos for q-tile tau: sub = d // 2 rows,
                # htiles (2j, 2j+1) output columns
                sub, jp = divmod(d, 2)
                A = A_tiles[tau % 4]
                i = tau * (QT // P) + sub
                if jp == 0:
                    stg_cur[0] = p2st.tile([P, H], BF16, tag="stg",
                                           name=f"stg{tau}_{sub}")
                for k in range(2):
                    j = 2 * jp + k
                    po = p2po.tile([P, QT], F32, tag="po",
                                   name=f"po{tau}_{d}_{k}")
                    for oc in range(HPC):
                        nc.tensor.matmul(
                            po,
                            lhsT=A[:, oc * QT + sub * P:oc * QT + (sub + 1) * P],
                            rhs=woT_sb[:, oc, j * QT:(j + 1) * QT],
                            start=(oc == 0), stop=(oc == HPC - 1),
                        )
                    # PSUM drain: 1/3 ACT, 2/3 DVE (gpsimd cannot read
                    # PSUM; ACT is near its exp-paced ceiling late)
                    dst = stg_cur[0][:, j * QT:(j + 1) * QT]
                    ucount[0] += 1
                    if ucount[0] % 3 == 0:
                        nc.scalar.activation(
                            dst, po, mybir.ActivationFunctionType.Copy
                        )
                    else:
                        nc.vector.tensor_copy(dst, po)
                if jp == 1:
                    nc.sync.dma_start(out[i * P:(i + 1) * P, :], stg_cur[0])

            for t in range(NQT):
                q0 = t * QT
                nch = 4 * t + 4
                # host 2t pending o_proj duos in this tile's chunk loop:
                # early tiles have little ACT/DVE slack, late tiles much
                nduo = min(2 * t, len(pending))
                pv_ps = p2pv.tile([P, 2 * QT], F32, tag="pv", name=f"pv{t}")
                acc2 = [
                    p2acc.tile([P, 2 * QT], BF16, tag=f"acc{a}",
                               name=f"acc{a}_{t}")
                    for a in range(2)
                ]
                for c in range(nch):
                    first = c == 0
                    last = c == nch - 1
                    # diagonal chunk j covers only q >= 128j within the
                    # tile: narrow scores+exp to that range for j >= 2
                    # (not worth the extra exp instruction for j == 1);
                    # the full-width mask multiply zeroes the unwritten
                    # region of e, so PV/acc read zeros there.
                    j = c - 4 * t if c >= 4 * t else -1
                    qoff = 128 * j if j >= 2 else 0
                    sc = p2sc.tile([P, 2 * QT], F32, tag="sc")
                    for h in range(HPC):
                        nc.tensor.matmul(
                            sc[:, h * QT + qoff:(h + 1) * QT],
                            lhsT=KT_sb[:, h, c * P:(c + 1) * P],
                            rhs=QT_sb[:, h, q0 + qoff:q0 + QT],
                            start=True, stop=True,
                        )
                    e = p2e.tile([P, 2 * QT], BF16, tag="e")
                    if qoff:
                        for h in range(HPC):
                            nc.scalar.activation(
                                e[:, h * QT + qoff:(h + 1) * QT],
                                sc[:, h * QT + qoff:(h + 1) * QT],
                                mybir.ActivationFunctionType.Exp, scale=SCALE,
                            )
                    else:
                        nc.scalar.activation(
                            e, sc, mybir.ActivationFunctionType.Exp, scale=SCALE
                        )
                    if j >= 0:
                        nc.vector.tensor_mul(e, e, masks_sb[:, j, :])
                    # softmax denominator accumulation (k lives on
                    # partitions): two alternating bf16 accumulators so the
                    # serial add chain is 2x shorter; folded across
                    # partitions once per q-tile below.
                    acc = acc2[c % 2]
                    if c < 2:
                        nc.vector.tensor_copy(acc, e)
                    else:
                        nc.vector.tensor_add(acc, acc, e)
                    for h in range(HPC):
                        nc.tensor.matmul(
                            pv_ps[:, h * QT:(h + 1) * QT],
                            lhsT=V_sb[:, c, h * P:(h + 1) * P],
                            rhs=e[:, h * QT:(h + 1) * QT],
                            start=first, stop=last,
                        )
                    # interleave pending o_proj duos so the PE has work
                    # while exp paces the chunk loop; floor-spread leaves
                    # the trailing chunks covered so the duos pad the
                    # fold/recip/norm chain at the tile boundary
                    if nduo and c >= 1:
                        k = min(nduo // (nch - c), nduo)
                        for _ in range(k):
                            oproj_duo(*pending.pop(0))
                        nduo -= k
                while nduo:
                    oproj_duo(*pending.pop(0))
                    nduo -= 1

                # fold partition sums -> broadcast [128, q] per head; the
                # folds live in po-pool slots so the sc pool's rotation is
                # not held hostage by the reciprocal below
                rec = p2rec.tile([P, 2 * QT], F32, tag="rec")
                for h in range(HPC):
                    fold = p2po.tile([P, QT], F32, tag="po", name=f"fold{t}_{h}")
                    for a in range(2):
                        nc.tensor.matmul(
                            fold,
                            lhsT=ones_sb,
                            rhs=acc2[a][:, h * QT:(h + 1) * QT],
                            start=(a == 0), stop=(a == 1),
                        )
                    nc.vector.reciprocal_approx_fast(
                        rec[:, h * QT:(h + 1) * QT], fold
                    )
                A = p2a.tile([P, 2 * QT], BF16, tag="A", name=f"A{t}")
                A_tiles[t % 4] = A
                nc.vector.tensor_mul(A, pv_ps, rec)
                pending.extend((t, d) for d in range(8))

            while pending:
                oproj_duo(*pending.pop(0))


def build_program():
    """Build + compile the Bass program (same program for all 8 cores)."""
    global _PROGRAM
    if _PROGRAM is not None:
        return _PROGRAM
    nc = bacc.Bacc(
        "TRN2", target_bir_lowering=False, debug=False, enable_asserts=False
    )
    with tile.TileContext(nc) as tc:
        _build_body(tc)
    nc.compile()
    _PROGRAM = nc
    return nc


def make_in_maps(hidden_states, w_qkv, w_o):
    import ml_dtypes

    x = np.asarray(hidden_states, dtype=np.float32).reshape(S, H)
    w = np.asarray(w_qkv, dtype=np.float32)
    wo = np.asarray(w_o, dtype=np.float32)

    xT = np.ascontiguousarray(x.T).astype(ml_dtypes.bfloat16)    # [2048, 4096]
    # [st, p, t, s]: per (st, p) the data is one contiguous 16 KiB run
    xTt = np.ascontiguousarray(
        xT.reshape(NHT, P, NQT, ST1).transpose(2, 1, 0, 3)
    )

    # rope tables, [128, 2, 8, 1024]: rows 0:64 and 64:128 both hold the
    # [64, S] table so the doubled layout lines up with [real; imag] dims;
    # per s-tile the 512-wide block is repeated twice so one [128, 1024]
    # DVE op covers both heads.
    e = np.arange(0, DH, 2, dtype=np.float32) / np.float32(DH)
    inv_freq = (1.0 / np.power(np.float32(10000.0), e)).astype(np.float32)
    t = np.arange(S, dtype=np.float32)
    freqs = np.outer(t, inv_freq).astype(np.float32)     # [S, 64]
    cosT = np.cos(freqs).T                               # [64, S]
    sinT = np.sin(freqs).T
    tabs = np.empty((P, 2, S), dtype=np.float32)
    tabs[0:64, 0] = cosT
    tabs[64:128, 0] = cosT
    tabs[0:64, 1] = sinT
    tabs[64:128, 1] = sinT
    rope = np.empty((P, 2, NQT, 2 * ST1), dtype=np.float32)
    for st in range(NQT):
        blk = tabs[:, :, st * ST1:(st + 1) * ST1]
        rope[:, :, st, 0:ST1] = blk
        rope[:, :, st, ST1:2 * ST1] = blk
    rope = rope.reshape(P, 2, NQT * 2 * ST1)

    # signed half-swap permutation: (J.T @ z)[d] = -z[64+d], [64+d] = +z[d]
    swapj = np.zeros((P, P), dtype=ml_dtypes.bfloat16)
    for d in range(64):
        swapj[64 + d, d] = -1.0
        swapj[d, 64 + d] = 1.0

    # diagonal-block masks [4, 128, 1024]: chunk at k0 = q0 + 128j keeps
    # (ki, qi) iff qi >= ki + 128j; tiled twice along q for the 2-head tile.
    ki = np.arange(P)[:, None]
    qi = np.arange(QT)[None, :]
    masks = np.empty((4, P, 2 * QT), dtype=ml_dtypes.bfloat16)
    for j in range(4):
        m = (qi >= ki + 128 * j).astype(ml_dtypes.bfloat16)
        masks[j] = np.concatenate([m, m], axis=1)

    in_maps = []
    for c in range(NCORES):
        r0 = c * OLOC
        w_loc = np.concatenate(
            [
                w[r0:r0 + OLOC],
                w[NH * DH + r0:NH * DH + r0 + OLOC],
                w[2 * NH * DH + r0:2 * NH * DH + r0 + OLOC],
            ],
            axis=0,
        )                                                # [768, 2048]
        wqkvT_c = w_loc.T.astype(ml_dtypes.bfloat16)     # [2048, 768]
        wqt_c = np.ascontiguousarray(
            wqkvT_c.reshape(NHT, P, 3 * OLOC).transpose(1, 0, 2)
        )                                                # [128, 16, 768]
        woT_c = np.ascontiguousarray(
            wo[:, r0:r0 + OLOC].T
        ).astype(ml_dtypes.bfloat16)                     # [256, 2048]
        in_maps.append(
            {
                "xTt": xTt,
                "wqt": wqt_c,
                "woT": woT_c,
                "rope": rope,
                "swapj": swapj,
                "onesin": np.ones((P, P), dtype=ml_dtypes.bfloat16),
                "masks": masks,
            }
        )
    return in_maps


def run_cores(in_maps, trace=False, **kwargs):
    nc = build_program()
    return run_bass_kernel_spmd(
        nc, in_maps, list(range(NCORES)), trace=trace, **kwargs
    )


def kernel(hidden_states, w_qkv, w_o):
    in_maps = make_in_maps(hidden_states, w_qkv, w_o)
    res = run_cores(in_maps)
    acc = res.results[0]["out"].astype(np.float32)
    for c in range(1, NCORES):
        acc = acc + res.results[c]["out"].astype(np.float32)
    return acc.reshape(1, S, H)


# revision 40
# speedup vs baseline: 1.1803x; 1.1803x over previous
"""Trainium2 Bass kernel for a 16-head causal self-attention block.

Reference computation (B=1, S=4096, H=2048, 16 heads x 128 dim, fp32):
    qkv = x @ w_qkv.T            # [S, 6144]
    q, k = rope(q), rope(k)      # half-split rope
    attn = causal_softmax(q k^T / sqrt(128)) @ v
    out  = attn @ w_o.T          # [S, 2048]

Sharding: tensor-parallel over heads.  Each of the 8 cores owns 2 heads:
it computes its slice of the QKV projection (768 rows), attention for its
2 heads, and a partial o_proj ([S, 2048], bf16); the host sums the 8
partials.

Dataflow per core (matmul operands bf16, accumulation/softmax fp32):
  phase 1 (QKV+rope), per s-tile of 512: one batched x^T DMA group per
    tile, then three matmul sweeps (V, Q, K) rotating over three PSUM
    slots so each sweep's epilogue drains while the next computes.  RoPE
    is fused into the Q/K epilogues; rotate_half is a signed-permutation
    matmul on the PE; cos/sin tables are pre-doubled so one [128,1024]
    DVE op covers both heads.
  phase 2 (attention), per q-tile of 512: loop causal k-chunks of 128:
    scoresT[k,q] -> ACT exp (both heads in one [128,1024] instr) ->
    diagonal-mask multiply -> PV matmul, with bf16 softmax-denominator
    accumulators (two alternating, halves the serial add chain); per
    q-tile a ones-matmul folds the partition sums, fast-reciprocal +
    one multiply normalizes attn^T.  o_proj for q-tile t-1 is
    interleaved into tile t's chunk loop (PE absorbs the exp-paced
    slack); its PSUM drains go to the otherwise-idle GpSimd engine.
"""

import numpy as np

import concourse.bass as bass
import concourse.mybir as mybir
import concourse.tile as tile
from concourse import bacc
from concourse.bass_utils import run_bass_kernel_spmd

F32 = mybir.dt.float32
BF16 = mybir.dt.bfloat16

S = 4096
H = 2048
DH = 128
NH = 16
NCORES = 8
HPC = NH // NCORES          # 2 heads per core
OLOC = HPC * DH             # 256 local o-channels per q/k/v group
P = 128
ST1 = 512                   # phase-1 s-tile width
NHT = H // P                # 16 h-chunks
QT = 512                    # phase-2 q-tile width
NQT = S // QT               # 8 q-tiles
NKC = S // P                # 32 k-chunks
SCALE = 1.0 / float(np.sqrt(np.float32(DH)))

_PROGRAM = None


def _build_body(tc):
    nc = tc.nc

    # x and w_qkv are host-retiled so each partition's chunk is one long
    # contiguous run (big DMA descriptors: ~25ns fixed cost per descriptor)
    xTt = nc.dram_tensor("xTt", [NQT, P, NHT, ST1], BF16, kind="ExternalInput").ap()
    wqt = nc.dram_tensor("wqt", [P, NHT, 3 * OLOC], BF16, kind="ExternalInput").ap()
    woT = nc.dram_tensor("woT", [OLOC, H], BF16, kind="ExternalInput").ap()
    rope = nc.dram_tensor("rope", [P, 2, NQT * 2 * ST1], F32, kind="ExternalInput").ap()
    swapj = nc.dram_tensor("swapj", [P, P], BF16, kind="ExternalInput").ap()
    onesin = nc.dram_tensor("onesin", [P, P], BF16, kind="ExternalInput").ap()
    masks = nc.dram_tensor("masks", [4, P, 2 * QT], BF16, kind="ExternalInput").ap()
    out = nc.dram_tensor("out", [S, H], BF16, kind="ExternalOutput").ap()

    woT_v = woT.rearrange("(t p) h -> p t h", p=P)      # [128, 2, 2048]

    with tc.tile_pool(name="resident", bufs=1) as resident:
        # d-major Q^T/K^T: [128 d, head, s]; s-major V: [128 s, k-chunk, 256]
        QT_sb = resident.tile([P, HPC, S], BF16)
        KT_sb = resident.tile([P, HPC, S], BF16)
        V_sb = resident.tile([P, NKC, OLOC], BF16)
        wT_sb = resident.tile([P, NHT, 3 * OLOC], BF16)
        woT_sb = resident.tile([P, HPC, H], BF16)
        masks_sb = resident.tile([P, 4, 2 * QT], BF16)
        ones_sb = resident.tile([P, P], BF16)
        J_sb = resident.tile([P, P], BF16)

        # All phase-1 inputs go on the sync queue in consumption order so
        # the first-needed transfers get the full DMA bandwidth instead of
        # fair-sharing it with later ones (issue spacing staggers them).
        nc.gpsimd.dma_start(J_sb, swapj)

        # ---------------- phase 1: QKV projection + rope ----------------
        with (
            tc.tile_pool(name="p1x", bufs=2) as p1x,
            tc.tile_pool(name="p1tab", bufs=2) as p1tab,
            tc.tile_pool(name="p1t1", bufs=2) as p1t1,
            tc.tile_pool(name="p1t2", bufs=2) as p1t2,
            tc.tile_pool(name="p1ps", bufs=3, space="PSUM") as p1ps,
            tc.tile_pool(name="p1rot", bufs=1, space="PSUM") as p1rot,
        ):
            def rope_block(blk, dst, s0, cos, sin):
                # blk: [128, 1024] PSUM (both heads); dst: QT_sb or KT_sb
                t1 = p1t1.tile([P, 2 * ST1], F32, tag="t1")
                t2 = p1t2.tile([P, 2 * ST1], BF16, tag="t2")
                nc.vector.tensor_mul(t1, blk, cos)
                nc.vector.tensor_mul(t2, blk, sin)
                rot = p1rot.tile([P, 2 * ST1], F32, tag="rot")
                nc.tensor.matmul(rot[:, 0:ST1], lhsT=J_sb, rhs=t2[:, 0:ST1],
                                 start=True, stop=True)
                nc.tensor.matmul(rot[:, ST1:2 * ST1], lhsT=J_sb,
                                 rhs=t2[:, ST1:2 * ST1], start=True, stop=True)
                for h in range(HPC):
                    nc.vector.tensor_add(
                        dst[:, h, s0:s0 + ST1],
                        t1[:, h * ST1:(h + 1) * ST1],
                        rot[:, h * ST1:(h + 1) * ST1],
                    )

            for st in range(S // ST1):
                s0 = st * ST1
                xt = p1x.tile([P, NHT, ST1], BF16, tag="xt", name=f"xt{st}")
                if st == 0:
                    # fine-grained interleave for the startup-critical tile
                    for g in range(8):
                        nc.sync.dma_start(
                            wT_sb[:, 2 * g:2 * (g + 1), :],
                            wqt[:, 2 * g:2 * (g + 1), :],
                        )
                        nc.sync.dma_start(
                            xt[:, 2 * g:2 * (g + 1), :],
                            xTt[st, :, 2 * g:2 * (g + 1), :],
                        )
                else:
                    for g in range(4):
                        nc.sync.dma_start(
                            xt[:, 4 * g:4 * (g + 1), :],
                            xTt[st, :, 4 * g:4 * (g + 1), :],
                        )
                tab = p1tab.tile([P, 2, 2 * ST1], F32, tag="tab")
                nc.sync.dma_start(
                    tab, rope[:, :, st * 2 * ST1:(st + 1) * 2 * ST1]
                )
                if st == 2:
                    # phase-2-only tensors: issued on the scalar queue here,
                    # behind two tiles of V-drain copies, so their traffic
                    # cannot crowd out the startup-critical loads above
                    nc.scalar.dma_start(ones_sb, onesin)
                    nc.scalar.dma_start(masks_sb, masks.rearrange("j p q -> p j q"))
                    nc.scalar.dma_start(woT_sb, woT_v)
                cos = tab[:, 0, :]
                sin = tab[:, 1, :]

                # --- V sweep ---
                # [128, 1024]: s-subs 0,1 share zero-region/bank 0; 2,3 share 1
                pv = p1ps.tile([P, 2 * ST1], F32, tag="qkv", name=f"pv{st}")
                for ht in range(NHT):
                    for sub in range(ST1 // P):
                        nc.tensor.matmul(
                            pv[:, sub * OLOC:(sub + 1) * OLOC],
                            lhsT=xt[:, ht, sub * P:(sub + 1) * P],
                            rhs=wT_sb[:, ht, 2 * OLOC:3 * OLOC],
                            start=(ht == 0) and sub % 2 == 0,
                            stop=(ht == NHT - 1) and sub % 2 == 1,
                        )
                for sub in range(ST1 // P):
                    nc.scalar.activation(
                        V_sb[:, st * (ST1 // P) + sub, :],
                        pv[:, sub * OLOC:(sub + 1) * OLOC],
                        mybir.ActivationFunctionType.Copy,
                    )

                # --- Q sweep + rope ---
                pq = p1ps.tile([P, 2 * ST1], F32, tag="qkv", name=f"pq{st}")
                for ht in range(NHT):
                    for h in range(HPC):
                        nc.tensor.matmul(
                            pq[:, h * ST1:(h + 1) * ST1],
                            lhsT=wT_sb[:, ht, h * P:(h + 1) * P],
                            rhs=xt[:, ht, :],
                            start=ht == 0, stop=ht == NHT - 1,
                        )
                rope_block(pq, QT_sb, s0, cos, sin)

                # --- K sweep + rope ---
                pk = p1ps.tile([P, 2 * ST1], F32, tag="qkv", name=f"pk{st}")
                for ht in range(NHT):
                    for h in range(HPC):
                        nc.tensor.matmul(
                            pk[:, h * ST1:(h + 1) * ST1],
                            lhsT=wT_sb[:, ht, OLOC + h * P:OLOC + (h + 1) * P],
                            rhs=xt[:, ht, :],
                            start=ht == 0, stop=ht == NHT - 1,
                        )
                rope_block(pk, KT_sb, s0, cos, sin)

        # ---------------- phase 2: attention + interleaved o_proj ---------
        with (
            tc.tile_pool(name="p2e", bufs=3) as p2e,
            tc.tile_pool(name="p2acc", bufs=2) as p2acc,
            tc.tile_pool(name="p2rec", bufs=2) as p2rec,
            tc.tile_pool(name="p2a", bufs=4) as p2a,
            tc.tile_pool(name="p2st", bufs=3) as p2st,
            tc.tile_pool(name="p2sc", bufs=2, space="PSUM") as p2sc,
            tc.tile_pool(name="p2pv", bufs=1, space="PSUM") as p2pv,
            tc.tile_pool(name="p2po", bufs=2, space="PSUM") as p2po,
        ):
            A_tiles = [None] * 4            # A_tiles[t % 4] = attnT of tile t
            stg_cur = [None]
            pending = []                    # (tau, d) o_proj duos not yet issued
            ucount = [0]

            def oproj_duo(tau, d):
                # one of 8 o_proj duos for q-tile tau: sub = d // 2 rows,
                # htiles (2j, 2j+1) output columns
                sub, jp = divmod(d, 2)
                A = A_tiles[tau % 4]
                i = tau * (QT // P) + sub
                if jp == 0:
                    stg_cur[0] = p2st.tile([P, H], BF16, tag="stg",
                                           name=f"stg{tau}_{sub}")
                for k in range(2):
                    j = 2 * jp + k
                    po = p2po.tile([P, QT], F32, tag="po",
                                   name=f"po{tau}_{d}_{k}")
                    for oc in range(HPC):
                        nc.tensor.matmul(
                            po,
                            lhsT=A[:, oc * QT + sub * P:oc * QT + (sub + 1) * P],
                            rhs=woT_sb[:, oc, j * QT:(j + 1) * QT],
                            start=(oc == 0), stop=(oc == HPC - 1),
                        )
                    # PSUM drain: 1/3 ACT, 2/3 DVE (gpsimd cannot read
                    # PSUM; ACT is near its exp-paced ceiling late)
                    dst = stg_cur[0][:, j * QT:(j + 1) * QT]
                    ucount[0] += 1
                    if ucount[0] % 3 == 0:
                        nc.scalar.activation(
                            dst, po, mybir.ActivationFunctionType.Copy
                        )
                    else:
                        nc.vector.tensor_copy(dst, po)
                if jp == 1:
                    nc.sync.dma_start(out[i * P:(i + 1) * P, :], stg_cur[0])

            for t in range(NQT):
                q0 = t * QT
                nch = 4 * t + 4
                # host 2t pending o_proj duos in this tile's chunk loop:
                # early tiles have little ACT/DVE slack, late tiles much
                nduo = min(2 * t, len(pending))
                pv_ps = p2pv.tile([P, 2 * QT], F32, tag="pv", name=f"pv{t}")
                acc2 = [
                    p2acc.tile([P, 2 * QT], BF16, tag=f"acc{a}",
                               name=f"acc{a}_{t}")
                    for a in range(2)
                ]
                for c in range(nch):
                    first = c == 0
                    last = c == nch - 1
                    # diagonal chunk j covers only q >= 128j within the
                    # tile: narrow scores+exp to that range for j >= 2
                    # (not worth the extra exp instruction for j == 1);
                    # the full-width mask multiply zeroes the unwritten
                    # region of e, so PV/acc read zeros there.
                    j = c - 4 * t if c >= 4 * t else -1
                    qoff = 128 * j if j >= 2 else 0
                    sc = p2sc.tile([P, 2 * QT], F32, tag="sc")
                    for h in range(HPC):
                        nc.tensor.matmul(
                            sc[:, h * QT + qoff:(h + 1) * QT],
                            lhsT=KT_sb[:, h, c * P:(c + 1) * P],
                            rhs=QT_sb[:, h, q0 + qoff:q0 + QT],
                            start=True, stop=True,
                        )
                    e = p2e.tile([P, 2 * QT], BF16, tag="e")
                    if qoff:
                        for h in range(HPC):
                            nc.scalar.activation(
                                e[:, h * QT + qoff:(h + 1) * QT],
                                sc[:, h * QT + qoff:(h + 1) * QT],
                                mybir.ActivationFunctionType.Exp, scale=SCALE,
                            )
                    else:
                        nc.scalar.activation(
                            e, sc, mybir.ActivationFunctionType.Exp, scale=SCALE
                        )
                    if j >= 0:
                        nc.vector.tensor_mul(e, e, masks_sb[:, j, :])
                    # softmax denominator accumulation (k lives on
                    # partitions): two alternating bf16 accumulators so the
                    # serial add chain is 2x shorter; folded across
                    # partitions once per q-tile below.
                    acc = acc2[c % 2]
                    if c < 2:
                        nc.vector.tensor_copy(acc, e)
                    else:
                        nc.vector.tensor_add(acc, acc, e)
                    for h in range(HPC):
                        nc.tensor.matmul(
                            pv_ps[:, h * QT:(h + 1) * QT],
                            lhsT=V_sb[:, c, h * P:(h + 1) * P],
                            rhs=e[:, h * QT:(h + 1) * QT],
                            start=first, stop=last,
                        )
                    # interleave pending o_proj duos so the PE has work
                    # while exp paces the chunk loop; floor-spread leaves
                    # the trailing chunks covered so the duos pad the
                    # fold/recip/norm chain at the tile boundary
                    if nduo and c >= 1:
                        k = min(nduo // (nch - c), nduo)
                        for _ in range(k):
                            oproj_duo(*pending.pop(0))
                        nduo -= k
                while nduo:
                    oproj_duo(*pending.pop(0))
                    nduo -= 1

                # fold partition sums -> broadcast [128, q] per head; the
                # folds live in po-pool slots so the sc pool's rotation is
                # not held hostage by the reciprocal below
                rec = p2rec.tile([P, 2 * QT], F32, tag="rec")
                for h in range(HPC):
                    fold = p2po.tile([P, QT], F32, tag="po", name=f"fold{t}_{h}")
                    for a in range(2):
                        nc.tensor.matmul(
                            fold,
                            lhsT=ones_sb,
                            rhs=acc2[a][:, h * QT:(h + 1) * QT],
                            start=(a == 0), stop=(a == 1),
                        )
                    nc.vector.reciprocal_approx_fast(
                        rec[:, h * QT:(h + 1) * QT], fold
                    )
                A = p2a.tile([P, 2 * QT], BF16, tag="A", name=f"A{t}")
                A_tiles[t % 4] = A
                nc.vector.tensor_mul(A, pv_ps, rec)
                pending.extend((t, d) for d in range(8))

            while pending:
                oproj_duo(*pending.pop(0))


def build_program():
    """Build + compile the Bass program (same program for all 8 cores)."""
    global _PROGRAM
    if _PROGRAM is not None:
        return _PROGRAM
    nc = bacc.Bacc(
        "TRN2", target_bir_lowering=False, debug=False, enable_asserts=False
    )
    with tile.TileContext(nc) as tc:
        _build_body(tc)
    nc.compile()
    _PROGRAM = nc
    return nc


def make_in_maps(hidden_states, w_qkv, w_o):
    import ml_dtypes

    x = np.asarray(hidden_states, dtype=np.float32).reshape(S, H)
    w = np.asarray(w_qkv, dtype=np.float32)
    wo = np.asarray(w_o, dtype=np.float32)

    xT = np.ascontiguousarray(x.T).astype(ml_dtypes.bfloat16)    # [2048, 4096]
    # [st, p, t, s]: per (st, p) the data is one contiguous 16 KiB run
    xTt = np.ascontiguousarray(
        xT.reshape(NHT, P, NQT, ST1).transpose(2, 1, 0, 3)
    )

    # rope tables, [128, 2, 8, 1024]: rows 0:64 and 64:128 both hold the
    # [64, S] table so the doubled layout lines up with [real; imag] dims;
    # per s-tile the 512-wide block is repeated twice so one [128, 1024]
    # DVE op covers both heads.
    e = np.arange(0, DH, 2, dtype=np.float32) / np.float32(DH)
    inv_freq = (1.0 / np.power(np.float32(10000.0), e)).astype(np.float32)
    t = np.arange(S, dtype=np.float32)
    freqs = np.outer(t, inv_freq).astype(np.float32)     # [S, 64]
    cosT = np.cos(freqs).T                               # [64, S]
    sinT = np.sin(freqs).T
    tabs = np.empty((P, 2, S), dtype=np.float32)
    tabs[0:64, 0] = cosT
    tabs[64:128, 0] = cosT
    tabs[0:64, 1] = sinT
    tabs[64:128, 1] = sinT
    rope = np.empty((P, 2, NQT, 2 * ST1), dtype=np.float32)
    for st in range(NQT):
        blk = tabs[:, :, st * ST1:(st + 1) * ST1]
        rope[:, :, st, 0:ST1] = blk
        rope[:, :, st, ST1:2 * ST1] = blk
    rope = rope.reshape(P, 2, NQT * 2 * ST1)

    # signed half-swap permutation: (J.T @ z)[d] = -z[64+d], [64+d] = +z[d]
    swapj = np.zeros((P, P), dtype=ml_dtypes.bfloat16)
    for d in range(64):
        swapj[64 + d, d] = -1.0
        swapj[d, 64 + d] = 1.0

    # diagonal-block masks [4, 128, 1024]: chunk at k0 = q0 + 128j keeps
    # (ki, qi) iff qi >= ki + 128j; tiled twice along q for the 2-head tile.
    ki = np.arange(P)[:, None]
    qi = np.arange(QT)[None, :]
    masks = np.empty((4, P, 2 * QT), dtype=ml_dtypes.bfloat16)
    for j in range(4):
        m = (qi >= ki + 128 * j).astype(ml_dtypes.bfloat16)
        masks[j] = np.concatenate([m, m], axis=1)

    in_maps = []
    for c in range(NCORES):
        r0 = c * OLOC
        w_loc = np.concatenate(
            [
                w[r0:r0 + OLOC],
                w[NH * DH + r0:NH * DH + r0 + OLOC],
                w[2 * NH * DH + r0:2 * NH * DH + r0 + OLOC],
            ],
            axis=0,
        )                                                # [768, 2048]
        wqkvT_c = w_loc.T.astype(ml_dtypes.bfloat16)     # [2048, 768]
        wqt_c = np.ascontiguousarray(
            wqkvT_c.reshape(NHT, P, 3 * OLOC).transpose(1, 0, 2)
        )                                                # [128, 16, 768]
        woT_c = np.ascontiguousarray(
            wo[:, r0:r0 + OLOC].T
        ).astype(ml_dtypes.bfloat16)                     # [256, 2048]
        in_maps.append(
            {
                "xTt": xTt,
                "wqt": wqt_c,
                "woT": woT_c,
                "rope": rope,
                "swapj": swapj,
                "onesin": np.ones((P, P), dtype=ml_dtypes.bfloat16),
                "masks": masks,
            }
        )
    return in_maps


def run_cores(in_maps, trace=False, **kwargs):
    nc = build_program()
    return run_bass_kernel_spmd(
        nc, in_maps, list(range(NCORES)), trace=trace, **kwargs
    )


def kernel(hidden_states, w_qkv, w_o):
    in_maps = make_in_maps(hidden_states, w_qkv, w_o)
    res = run_cores(in_maps)
    acc = res.results[0]["out"].astype(np.float32)
    for c in range(1, NCORES):
        acc = acc + res.results[c]["out"].astype(np.float32)
    return acc.reshape(1, S, H)


# revision 43
# speedup vs baseline: 1.1956x; 1.0130x over previous
"""Trainium2 Bass kernel for a 16-head causal self-attention block.

Reference computation (B=1, S=4096, H=2048, 16 heads x 128 dim, fp32):
    qkv = x @ w_qkv.T            # [S, 6144]
    q, k = rope(q), rope(k)      # half-split rope
    attn = causal_softmax(q k^T / sqrt(128)) @ v
    out  = attn @ w_o.T          # [S, 2048]

Sharding: tensor-parallel over heads.  Each of the 8 cores owns 2 heads:
it computes its slice of the QKV projection (768 rows), attention for its
2 heads, and a partial o_proj ([S, 2048], bf16); the host sums the 8
partials.

Dataflow per core (matmul operands bf16, accumulation/softmax fp32):
  phase 1 (QKV+rope), per s-tile of 512: one batched x^T DMA group per
    tile, then three matmul sweeps (V, Q, K) rotating over three PSUM
    slots so each sweep's epilogue drains while the next computes.  RoPE
    is fused into the Q/K epilogues; rotate_half is a signed-permutation
    matmul on the PE; cos/sin tables are pre-doubled so one [128,1024]
    DVE op covers both heads.
  phase 2 (attention), per q-tile of 512: loop causal k-chunks of 128:
    scoresT[k,q] -> ACT exp (both heads in one [128,1024] instr) ->
    diagonal-mask multiply -> PV matmul, with bf16 softmax-denominator
    accumulators (two alternating, halves the serial add chain); per
    q-tile a ones-matmul folds the partition sums, fast-reciprocal +
    one multiply normalizes attn^T.  o_proj for q-tile t-1 is
    interleaved into tile t's chunk loop (PE absorbs the exp-paced
    slack); its PSUM drains go to the otherwise-idle GpSimd engine.
"""

import numpy as np

import concourse.bass as bass
import concourse.mybir as mybir
import concourse.tile as tile
from concourse import bacc
from concourse.bass_utils import run_bass_kernel_spmd

F32 = mybir.dt.float32
BF16 = mybir.dt.bfloat16

S = 4096
H = 2048
DH = 128
NH = 16
NCORES = 8
HPC = NH // NCORES          # 2 heads per core
OLOC = HPC * DH             # 256 local o-channels per q/k/v group
P = 128
ST1 = 512                   # phase-1 s-tile width
NHT = H // P                # 16 h-chunks
QT = 512                    # phase-2 q-tile width
NQT = S // QT               # 8 q-tiles
NKC = S // P                # 32 k-chunks
SCALE = 1.0 / float(np.sqrt(np.float32(DH)))

_PROGRAM = None


def _build_body(tc):
    nc = tc.nc

    # x and w_qkv are host-retiled so each partition's chunk is one long
    # contiguous run (big DMA descriptors: ~25ns fixed cost per descriptor)
    xTt = nc.dram_tensor("xTt", [NQT, P, NHT, ST1], BF16, kind="ExternalInput").ap()
    wqt = nc.dram_tensor("wqt", [P, NHT, 3 * OLOC], BF16, kind="ExternalInput").ap()
    woT = nc.dram_tensor("woT", [OLOC, H], BF16, kind="ExternalInput").ap()
    rope = nc.dram_tensor("rope", [P, 2, NQT * 2 * ST1], F32, kind="ExternalInput").ap()
    swapj = nc.dram_tensor("swapj", [P, P], BF16, kind="ExternalInput").ap()
    onesin = nc.dram_tensor("onesin", [P, P], BF16, kind="ExternalInput").ap()
    masks = nc.dram_tensor("masks", [4, P, 2 * QT], BF16, kind="ExternalInput").ap()
    out = nc.dram_tensor("out", [S, H], BF16, kind="ExternalOutput").ap()

    woT_v = woT.rearrange("(t p) h -> p t h", p=P)      # [128, 2, 2048]

    with tc.tile_pool(name="resident", bufs=1) as resident:
        # d-major Q^T/K^T: [128 d, head, s]; s-major V: [128 s, k-chunk, 256]
        QT_sb = resident.tile([P, HPC, S], BF16)
        KT_sb = resident.tile([P, HPC, S], BF16)
        V_sb = resident.tile([P, NKC, OLOC], BF16)
        wT_sb = resident.tile([P, NHT, 3 * OLOC], BF16)
        woT_sb = resident.tile([P, HPC, H], BF16)
        masks_sb = resident.tile([P, 4, 2 * QT], BF16)
        ones_sb = resident.tile([P, P], BF16)
        J_sb = resident.tile([P, P], BF16)

        # All phase-1 inputs go on the sync queue in consumption order so
        # the first-needed transfers get the full DMA bandwidth instead of
        # fair-sharing it with later ones (issue spacing staggers them).
        nc.gpsimd.dma_start(J_sb, swapj)

        # ---------------- phase 1: QKV projection + rope ----------------
        with (
            tc.tile_pool(name="p1x", bufs=2) as p1x,
            tc.tile_pool(name="p1tab", bufs=2) as p1tab,
            tc.tile_pool(name="p1t1", bufs=2) as p1t1,
            tc.tile_pool(name="p1t2", bufs=2) as p1t2,
            tc.tile_pool(name="p1ps", bufs=3, space="PSUM") as p1ps,
            tc.tile_pool(name="p1rot", bufs=1, space="PSUM") as p1rot,
        ):
            def rope_block(blk, dst, s0, cos, sin):
                # blk: [128, 1024] PSUM (both heads); dst: QT_sb or KT_sb
                t1 = p1t1.tile([P, 2 * ST1], F32, tag="t1")
                t2 = p1t2.tile([P, 2 * ST1], BF16, tag="t2")
                nc.vector.tensor_mul(t1, blk, cos)
                nc.vector.tensor_mul(t2, blk, sin)
                rot = p1rot.tile([P, 2 * ST1], F32, tag="rot")
                nc.tensor.matmul(rot[:, 0:ST1], lhsT=J_sb, rhs=t2[:, 0:ST1],
                                 start=True, stop=True)
                nc.tensor.matmul(rot[:, ST1:2 * ST1], lhsT=J_sb,
                                 rhs=t2[:, ST1:2 * ST1], start=True, stop=True)
                for h in range(HPC):
                    nc.vector.tensor_add(
                        dst[:, h, s0:s0 + ST1],
                        t1[:, h * ST1:(h + 1) * ST1],
                        rot[:, h * ST1:(h + 1) * ST1],
                    )

            for st in range(S // ST1):
                s0 = st * ST1
                xt = p1x.tile([P, NHT, ST1], BF16, tag="xt", name=f"xt{st}")
                if st == 0:
                    # fine-grained interleave for the startup-critical tile
                    for g in range(8):
                        nc.sync.dma_start(
                            wT_sb[:, 2 * g:2 * (g + 1), :],
                            wqt[:, 2 * g:2 * (g + 1), :],
                        )
                        nc.sync.dma_start(
                            xt[:, 2 * g:2 * (g + 1), :],
                            xTt[st, :, 2 * g:2 * (g + 1), :],
                        )
                else:
                    for g in range(4):
                        nc.sync.dma_start(
                            xt[:, 4 * g:4 * (g + 1), :],
                            xTt[st, :, 4 * g:4 * (g + 1), :],
                        )
                tab = p1tab.tile([P, 2, 2 * ST1], F32, tag="tab")
                nc.sync.dma_start(
                    tab, rope[:, :, st * 2 * ST1:(st + 1) * 2 * ST1]
                )
                if st == 2:
                    # phase-2-only tensors: issued on the scalar queue here,
                    # behind two tiles of V-drain copies, so their traffic
                    # cannot crowd out the startup-critical loads above
                    nc.scalar.dma_start(ones_sb, onesin)
                    nc.scalar.dma_start(masks_sb, masks.rearrange("j p q -> p j q"))
                    nc.scalar.dma_start(woT_sb, woT_v)
                cos = tab[:, 0, :]
                sin = tab[:, 1, :]

                # --- V sweep ---
                # [128, 1024]: s-subs 0,1 share zero-region/bank 0; 2,3 share 1
                pv = p1ps.tile([P, 2 * ST1], F32, tag="qkv", name=f"pv{st}")
                for ht in range(NHT):
                    for sub in range(ST1 // P):
                        nc.tensor.matmul(
                            pv[:, sub * OLOC:(sub + 1) * OLOC],
                            lhsT=xt[:, ht, sub * P:(sub + 1) * P],
                            rhs=wT_sb[:, ht, 2 * OLOC:3 * OLOC],
                            start=(ht == 0) and sub % 2 == 0,
                            stop=(ht == NHT - 1) and sub % 2 == 1,
                        )
                for sub in range(ST1 // P):
                    nc.scalar.activation(
                        V_sb[:, st * (ST1 // P) + sub, :],
                        pv[:, sub * OLOC:(sub + 1) * OLOC],
                        mybir.ActivationFunctionType.Copy,
                    )

                # --- Q sweep + rope ---
                pq = p1ps.tile([P, 2 * ST1], F32, tag="qkv", name=f"pq{st}")
                for ht in range(NHT):
                    for h in range(HPC):
                        nc.tensor.matmul(
                            pq[:, h * ST1:(h + 1) * ST1],
                            lhsT=wT_sb[:, ht, h * P:(h + 1) * P],
                            rhs=xt[:, ht, :],
                            start=ht == 0, stop=ht == NHT - 1,
                        )
                rope_block(pq, QT_sb, s0, cos, sin)

                # --- K sweep + rope ---
                pk = p1ps.tile([P, 2 * ST1], F32, tag="qkv", name=f"pk{st}")
                for ht in range(NHT):
                    for h in range(HPC):
                        nc.tensor.matmul(
                            pk[:, h * ST1:(h + 1) * ST1],
                            lhsT=wT_sb[:, ht, OLOC + h * P:OLOC + (h + 1) * P],
                            rhs=xt[:, ht, :],
                            start=ht == 0, stop=ht == NHT - 1,
                        )
                rope_block(pk, KT_sb, s0, cos, sin)

        # ---------------- phase 2: attention + interleaved o_proj ---------
        with (
            tc.tile_pool(name="p2e", bufs=3) as p2e,
            tc.tile_pool(name="p2acc", bufs=2) as p2acc,
            tc.tile_pool(name="p2rec", bufs=2) as p2rec,
            tc.tile_pool(name="p2a", bufs=4) as p2a,
            tc.tile_pool(name="p2st", bufs=3) as p2st,
            tc.tile_pool(name="p2pvs", bufs=2) as p2pvs,
            tc.tile_pool(name="p2sc", bufs=2, space="PSUM") as p2sc,
            tc.tile_pool(name="p2pv", bufs=1, space="PSUM") as p2pv,
            tc.tile_pool(name="p2po", bufs=2, space="PSUM") as p2po,
        ):
            A_tiles = [None] * 4            # A_tiles[t % 4] = attnT of tile t
            stg_cur = [None]
            pending = []                    # (tau, d) o_proj duos not yet issued
            ucount = [0]

            def oproj_duo(tau, d):
                # one of 8 o_proj duos for q-tile tau: sub = d // 2 rows,
                # htiles (2j, 2j+1) output columns
                sub, jp = divmod(d, 2)
                A = A_tiles[tau % 4]
                i = tau * (QT // P) + sub
                if jp == 0:
                    stg_cur[0] = p2st.tile([P, H], BF16, tag="stg",
                                           name=f"stg{tau}_{sub}")
                for k in range(2):
                    j = 2 * jp + k
                    po = p2po.tile([P, QT], F32, tag="po",
                                   name=f"po{tau}_{d}_{k}")
                    for oc in range(HPC):
                        nc.tensor.matmul(
                            po,
                            lhsT=A[:, oc * QT + sub * P:oc * QT + (sub + 1) * P],
                            rhs=woT_sb[:, oc, j * QT:(j + 1) * QT],
                            start=(oc == 0), stop=(oc == HPC - 1),
                        )
                    # PSUM drain: 1/3 ACT, 2/3 DVE (gpsimd cannot read
                    # PSUM; ACT is near its exp-paced ceiling late)
                    dst = stg_cur[0][:, j * QT:(j + 1) * QT]
                    ucount[0] += 1
                    if ucount[0] % 3 == 0:
                        nc.scalar.activation(
                            dst, po, mybir.ActivationFunctionType.Copy
                        )
                    else:
                        nc.vector.tensor_copy(dst, po)
                if jp == 1:
                    nc.sync.dma_start(out[i * P:(i + 1) * P, :], stg_cur[0])

            def make_epilogue(t, pvs, acc2):
                # fold/recip/norm for tile t, issued lazily inside tile
                # t+1's chunk loop so the boundary has no serial chain
                def epi():
                    rec = p2rec.tile([P, 2 * QT], F32, tag="rec",
                                     name=f"rec{t}")
                    for h in range(HPC):
                        fold = p2po.tile([P, QT], F32, tag="po",
                                         name=f"fold{t}_{h}")
                        for a in range(2):
                            nc.tensor.matmul(
                                fold,
                                lhsT=ones_sb,
                                rhs=acc2[a][:, h * QT:(h + 1) * QT],
                                start=(a == 0), stop=(a == 1),
                            )
                        nc.vector.reciprocal_approx_fast(
                            rec[:, h * QT:(h + 1) * QT], fold
                        )
                    A = p2a.tile([P, 2 * QT], BF16, tag="A", name=f"A{t}")
                    A_tiles[t % 4] = A
                    nc.vector.tensor_mul(A, pvs, rec)
                    pending.extend((t, d) for d in range(8))
                return epi

            epi_prev = None
            for t in range(NQT):
                q0 = t * QT
                nch = 4 * t + 4
                # host up to 2t pending o_proj duos in this tile's loop:
                # early tiles have little ACT/DVE slack, late tiles much
                cap = 2 * t
                issued = 0
                pv_ps = p2pv.tile([P, 2 * QT], F32, tag="pv", name=f"pv{t}")
                acc2 = [
                    p2acc.tile([P, 2 * QT], BF16, tag=f"acc{a}",
                               name=f"acc{a}_{t}")
                    for a in range(2)
                ]
                for c in range(nch):
                    first = c == 0
                    last = c == nch - 1
                    # diagonal chunk j covers only q >= 128j within the
                    # tile: narrow scores+exp to that range for j >= 2
                    # (not worth the extra exp instruction for j == 1);
                    # the full-width mask multiply zeroes the unwritten
                    # region of e, so PV/acc read zeros there.
                    j = c - 4 * t if c >= 4 * t else -1
                    qoff = 128 * j if j >= 2 else 0
                    sc = p2sc.tile([P, 2 * QT], F32, tag="sc")
                    for h in range(HPC):
                        nc.tensor.matmul(
                            sc[:, h * QT + qoff:(h + 1) * QT],
                            lhsT=KT_sb[:, h, c * P:(c + 1) * P],
                            rhs=QT_sb[:, h, q0 + qoff:q0 + QT],
                            start=True, stop=True,
                        )
                    e = p2e.tile([P, 2 * QT], BF16, tag="e")
                    if qoff:
                        for h in range(HPC):
                            nc.scalar.activation(
                                e[:, h * QT + qoff:(h + 1) * QT],
                                sc[:, h * QT + qoff:(h + 1) * QT],
                                mybir.ActivationFunctionType.Exp, scale=SCALE,
                            )
                    else:
                        nc.scalar.activation(
                            e, sc, mybir.ActivationFunctionType.Exp, scale=SCALE
                        )
                    if j >= 0:
                        nc.vector.tensor_mul(e, e, masks_sb[:, j, :])
                    # softmax denominator accumulation (k lives on
                    # partitions): two alternating bf16 accumulators so the
                    # serial add chain is 2x shorter; folded across
                    # partitions once per q-tile below.
                    acc = acc2[c % 2]
                    if c < 2:
                        nc.vector.tensor_copy(acc, e)
                    else:
                        nc.vector.tensor_add(acc, acc, e)
                    for h in range(HPC):
                        nc.tensor.matmul(
                            pv_ps[:, h * QT:(h + 1) * QT],
                            lhsT=V_sb[:, c, h * P:(h + 1) * P],
                            rhs=e[:, h * QT:(h + 1) * QT],
                            start=first, stop=last,
                        )
                    # previous tile's softmax epilogue becomes PE/DVE
                    # filler early in this tile's loop
                    if c == 1 and epi_prev is not None:
                        epi_prev()
                        epi_prev = None
                    # interleave pending o_proj duos so the PE has work
                    # while exp paces the chunk loop; floor-spread leaves
                    # the trailing chunks covered
                    if c >= 2 and issued < cap and pending:
                        k = min((cap - issued) // (nch - c),
                                cap - issued, len(pending))
                        for _ in range(k):
                            oproj_duo(*pending.pop(0))
                            issued += 1
                while issued < cap and pending:
                    oproj_duo(*pending.pop(0))
                    issued += 1

                # drain the unnormalized attn accumulator to SBUF so the
                # single PSUM pv buffer frees without waiting on
                # fold/recip/norm
                pvs = p2pvs.tile([P, 2 * QT], BF16, tag="pvs", name=f"pvs{t}")
                nc.vector.tensor_copy(pvs, pv_ps)
                epi = make_epilogue(t, pvs, acc2)
                if t < NQT - 1:
                    epi_prev = epi
                else:
                    epi()

            while pending:
                oproj_duo(*pending.pop(0))


def build_program():
    """Build + compile the Bass program (same program for all 8 cores)."""
    global _PROGRAM
    if _PROGRAM is not None:
        return _PROGRAM
    nc = bacc.Bacc(
        "TRN2", target_bir_lowering=False, debug=False, enable_asserts=False
    )
    with tile.TileContext(nc) as tc:
        _build_body(tc)
    nc.compile()
    _PROGRAM = nc
    return nc


def make_in_maps(hidden_states, w_qkv, w_o):
    import ml_dtypes

    x = np.asarray(hidden_states, dtype=np.float32).reshape(S, H)
    w = np.asarray(w_qkv, dtype=np.float32)
    wo = np.asarray(w_o, dtype=np.float32)

    xT = np.ascontiguousarray(x.T).astype(ml_dtypes.bfloat16)    # [2048, 4096]
    # [st, p, t, s]: per (st, p) the data is one contiguous 16 KiB run
    xTt = np.ascontiguousarray(
        xT.reshape(NHT, P, NQT, ST1).transpose(2, 1, 0, 3)
    )

    # rope tables, [128, 2, 8, 1024]: rows 0:64 and 64:128 both hold the
    # [64, S] table so the doubled layout lines up with [real; imag] dims;
    # per s-tile the 512-wide block is repeated twice so one [128, 1024]
    # DVE op covers both heads.
    e = np.arange(0, DH, 2, dtype=np.float32) / np.float32(DH)
    inv_freq = (1.0 / np.power(np.float32(10000.0), e)).astype(np.float32)
    t = np.arange(S, dtype=np.float32)
    freqs = np.outer(t, inv_freq).astype(np.float32)     # [S, 64]
    cosT = np.cos(freqs).T                               # [64, S]
    sinT = np.sin(freqs).T
    tabs = np.empty((P, 2, S), dtype=np.float32)
    tabs[0:64, 0] = cosT
    tabs[64:128, 0] = cosT
    tabs[0:64, 1] = sinT
    tabs[64:128, 1] = sinT
    rope = np.empty((P, 2, NQT, 2 * ST1), dtype=np.float32)
    for st in range(NQT):
        blk = tabs[:, :, st * ST1:(st + 1) * ST1]
        rope[:, :, st, 0:ST1] = blk
        rope[:, :, st, ST1:2 * ST1] = blk
    rope = rope.reshape(P, 2, NQT * 2 * ST1)

    # signed half-swap permutation: (J.T @ z)[d] = -z[64+d], [64+d] = +z[d]
    swapj = np.zeros((P, P), dtype=ml_dtypes.bfloat16)
    for d in range(64):
        swapj[64 + d, d] = -1.0
        swapj[d, 64 + d] = 1.0

    # diagonal-block masks [4, 128, 1024]: chunk at k0 = q0 + 128j keeps
    # (ki, qi) iff qi >= ki + 128j; tiled twice along q for the 2-head tile.
    ki = np.arange(P)[:, None]
    qi = np.arange(QT)[None, :]
    masks = np.empty((4, P, 2 * QT), dtype=ml_dtypes.bfloat16)
    for j in range(4):
        m = (qi >= ki + 128 * j).astype(ml_dtypes.bfloat16)
        masks[j] = np.concatenate([m, m], axis=1)

    in_maps = []
    for c in range(NCORES):
        r0 = c * OLOC
        w_loc = np.concatenate(
            [
                w[r0:r0 + OLOC],
                w[NH * DH + r0:NH * DH + r0 + OLOC],
                w[2 * NH * DH + r0:2 * NH * DH + r0 + OLOC],
            ],
            axis=0,
        )                                                # [768, 2048]
        wqkvT_c = w_loc.T.astype(ml_dtypes.bfloat16)     # [2048, 768]
        wqt_c = np.ascontiguousarray(
            wqkvT_c.reshape(NHT, P, 3 * OLOC).transpose(1, 0, 2)
        )                                                # [128, 16, 768]
        woT_c = np.ascontiguousarray(
            wo[:, r0:r0 + OLOC].T
        ).astype(ml_dtypes.bfloat16)                     # [256, 2048]
        in_maps.append(
            {
                "xTt": xTt,
                "wqt": wqt_c,
                "woT": woT_c,
                "rope": rope,
                "swapj": swapj,
                "onesin": np.ones((P, P), dtype=ml_dtypes.bfloat16),
                "masks": masks,
            }
        )
    return in_maps


def run_cores(in_maps, trace=False, **kwargs):
    nc = build_program()
    return run_bass_kernel_spmd(
        nc, in_maps, list(range(NCORES)), trace=trace, **kwargs
    )


def kernel(hidden_states, w_qkv, w_o):
    in_maps = make_in_maps(hidden_states, w_qkv, w_o)
    res = run_cores(in_maps)
    acc = res.results[0]["out"].astype(np.float32)
    for c in range(1, NCORES):
        acc = acc + res.results[c]["out"].astype(np.float32)
    return acc.reshape(1, S, H)


# revision 48
# speedup vs baseline: 1.1982x; 1.0022x over previous
"""Trainium2 Bass kernel for a 16-head causal self-attention block.

Reference computation (B=1, S=4096, H=2048, 16 heads x 128 dim, fp32):
    qkv = x @ w_qkv.T            # [S, 6144]
    q, k = rope(q), rope(k)      # half-split rope
    attn = causal_softmax(q k^T / sqrt(128)) @ v
    out  = attn @ w_o.T          # [S, 2048]

Sharding: tensor-parallel over heads.  Each of the 8 cores owns 2 heads:
it computes its slice of the QKV projection (768 rows), attention for its
2 heads, and a partial o_proj ([S, 2048], bf16); the host sums the 8
partials.

Dataflow per core (matmul operands bf16, accumulation/softmax fp32):
  phase 1 (QKV+rope), per s-tile of 512: one batched x^T DMA group per
    tile, then three matmul sweeps (V, Q, K) rotating over three PSUM
    slots so each sweep's epilogue drains while the next computes.  RoPE
    is fused into the Q/K epilogues; rotate_half is a signed-permutation
    matmul on the PE; cos/sin tables are pre-doubled so one [128,1024]
    DVE op covers both heads.
  phase 2 (attention), per q-tile of 512: loop causal k-chunks of 128:
    scoresT[k,q] -> ACT exp (both heads in one [128,1024] instr) ->
    diagonal-mask multiply -> PV matmul, with bf16 softmax-denominator
    accumulators (two alternating, halves the serial add chain); per
    q-tile a ones-matmul folds the partition sums, fast-reciprocal +
    one multiply normalizes attn^T.  o_proj for q-tile t-1 is
    interleaved into tile t's chunk loop (PE absorbs the exp-paced
    slack); its PSUM drains go to the otherwise-idle GpSimd engine.
"""

import numpy as np

import concourse.bass as bass
import concourse.mybir as mybir
import concourse.tile as tile
from concourse import bacc
from concourse.bass_utils import run_bass_kernel_spmd

F32 = mybir.dt.float32
BF16 = mybir.dt.bfloat16

S = 4096
H = 2048
DH = 128
NH = 16
NCORES = 8
HPC = NH // NCORES          # 2 heads per core
OLOC = HPC * DH             # 256 local o-channels per q/k/v group
P = 128
ST1 = 512                   # phase-1 s-tile width
NHT = H // P                # 16 h-chunks
QT = 512                    # phase-2 q-tile width
NQT = S // QT               # 8 q-tiles
NKC = S // P                # 32 k-chunks
SCALE = 1.0 / float(np.sqrt(np.float32(DH)))

_PROGRAM = None


def _build_body(tc):
    nc = tc.nc

    # x and w_qkv are host-retiled so each partition's chunk is one long
    # contiguous run (big DMA descriptors: ~25ns fixed cost per descriptor)
    xTt = nc.dram_tensor("xTt", [NQT, P, NHT, ST1], BF16, kind="ExternalInput").ap()
    wqt = nc.dram_tensor("wqt", [3, P, NHT, OLOC], BF16, kind="ExternalInput").ap()
    woT = nc.dram_tensor("woT", [OLOC, H], BF16, kind="ExternalInput").ap()
    rope = nc.dram_tensor("rope", [P, 2, NQT * 2 * ST1], F32, kind="ExternalInput").ap()
    swapj = nc.dram_tensor("swapj", [P, P], BF16, kind="ExternalInput").ap()
    onesin = nc.dram_tensor("onesin", [P, P], BF16, kind="ExternalInput").ap()
    masks = nc.dram_tensor("masks", [4, P, 2 * QT], BF16, kind="ExternalInput").ap()
    out = nc.dram_tensor("out", [S, H], BF16, kind="ExternalOutput").ap()

    woT_v = woT.rearrange("(t p) h -> p t h", p=P)      # [128, 2, 2048]

    with tc.tile_pool(name="resident", bufs=1) as resident:
        # d-major Q^T/K^T: [128 d, head, s]; s-major V: [128 s, k-chunk, 256]
        QT_sb = resident.tile([P, HPC, S], BF16)
        KT_sb = resident.tile([P, HPC, S], BF16)
        V_sb = resident.tile([P, NKC, OLOC], BF16)
        wT_sb = resident.tile([P, NHT, 3 * OLOC], BF16)
        woT_sb = resident.tile([P, HPC, H], BF16)
        masks_sb = resident.tile([P, 4, 2 * QT], BF16)
        ones_sb = resident.tile([P, P], BF16)
        J_sb = resident.tile([P, P], BF16)

        # All phase-1 inputs go on the sync queue in consumption order so
        # the first-needed transfers get the full DMA bandwidth instead of
        # fair-sharing it with later ones (issue spacing staggers them).
        nc.gpsimd.dma_start(J_sb, swapj)

        # ---------------- phase 1: QKV projection + rope ----------------
        with (
            tc.tile_pool(name="p1x", bufs=2) as p1x,
            tc.tile_pool(name="p1tab", bufs=2) as p1tab,
            tc.tile_pool(name="p1t1", bufs=2) as p1t1,
            tc.tile_pool(name="p1t2", bufs=2) as p1t2,
            tc.tile_pool(name="p1ps", bufs=3, space="PSUM") as p1ps,
            tc.tile_pool(name="p1rot", bufs=1, space="PSUM") as p1rot,
        ):
            def rope_block(blk, dst, s0, cos, sin):
                # blk: [128, 1024] PSUM (both heads); dst: QT_sb or KT_sb
                t1 = p1t1.tile([P, 2 * ST1], F32, tag="t1")
                t2 = p1t2.tile([P, 2 * ST1], BF16, tag="t2")
                nc.vector.tensor_mul(t1, blk, cos)
                nc.vector.tensor_mul(t2, blk, sin)
                rot = p1rot.tile([P, 2 * ST1], F32, tag="rot")
                nc.tensor.matmul(rot[:, 0:ST1], lhsT=J_sb, rhs=t2[:, 0:ST1],
                                 start=True, stop=True)
                nc.tensor.matmul(rot[:, ST1:2 * ST1], lhsT=J_sb,
                                 rhs=t2[:, ST1:2 * ST1], start=True, stop=True)
                for h in range(HPC):
                    nc.vector.tensor_add(
                        dst[:, h, s0:s0 + ST1],
                        t1[:, h * ST1:(h + 1) * ST1],
                        rot[:, h * ST1:(h + 1) * ST1],
                    )

            for st in range(S // ST1):
                s0 = st * ST1
                xt = p1x.tile([P, NHT, ST1], BF16, tag="xt", name=f"xt{st}")
                if st == 0:
                    # V columns gate the first sweep; Q/K columns follow
                    # behind the tile-0 x loads (g-major host layout keeps
                    # every one of these a large-descriptor transfer)
                    nc.sync.dma_start(
                        wT_sb[:, :, 2 * OLOC:3 * OLOC], wqt[2]
                    )
                for g in range(4):
                    nc.sync.dma_start(
                        xt[:, 4 * g:4 * (g + 1), :],
                        xTt[st, :, 4 * g:4 * (g + 1), :],
                    )
                if st == 0:
                    nc.sync.dma_start(wT_sb[:, :, 0:OLOC], wqt[0])
                    nc.sync.dma_start(wT_sb[:, :, OLOC:2 * OLOC], wqt[1])
                tab = p1tab.tile([P, 2, 2 * ST1], F32, tag="tab")
                nc.sync.dma_start(
                    tab, rope[:, :, st * 2 * ST1:(st + 1) * 2 * ST1]
                )
                if st == 2:
                    # phase-2-only tensors: issued on the scalar queue here,
                    # behind two tiles of V-drain copies, so their traffic
                    # cannot crowd out the startup-critical loads above
                    nc.scalar.dma_start(ones_sb, onesin)
                    nc.scalar.dma_start(masks_sb, masks.rearrange("j p q -> p j q"))
                    nc.scalar.dma_start(woT_sb, woT_v)
                cos = tab[:, 0, :]
                sin = tab[:, 1, :]

                # --- V sweep ---
                # [128, 1024]: s-subs 0,1 share zero-region/bank 0; 2,3 share 1
                pv = p1ps.tile([P, 2 * ST1], F32, tag="qkv", name=f"pv{st}")
                for ht in range(NHT):
                    for sub in range(ST1 // P):
                        nc.tensor.matmul(
                            pv[:, sub * OLOC:(sub + 1) * OLOC],
                            lhsT=xt[:, ht, sub * P:(sub + 1) * P],
                            rhs=wT_sb[:, ht, 2 * OLOC:3 * OLOC],
                            start=(ht == 0) and sub % 2 == 0,
                            stop=(ht == NHT - 1) and sub % 2 == 1,
                        )
                for sub in range(ST1 // P):
                    nc.scalar.activation(
                        V_sb[:, st * (ST1 // P) + sub, :],
                        pv[:, sub * OLOC:(sub + 1) * OLOC],
                        mybir.ActivationFunctionType.Copy,
                    )

                # --- Q sweep + rope ---
                pq = p1ps.tile([P, 2 * ST1], F32, tag="qkv", name=f"pq{st}")
                for ht in range(NHT):
                    for h in range(HPC):
                        nc.tensor.matmul(
                            pq[:, h * ST1:(h + 1) * ST1],
                            lhsT=wT_sb[:, ht, h * P:(h + 1) * P],
                            rhs=xt[:, ht, :],
                            start=ht == 0, stop=ht == NHT - 1,
                        )
                rope_block(pq, QT_sb, s0, cos, sin)

                # --- K sweep + rope ---
                pk = p1ps.tile([P, 2 * ST1], F32, tag="qkv", name=f"pk{st}")
                for ht in range(NHT):
                    for h in range(HPC):
                        nc.tensor.matmul(
                            pk[:, h * ST1:(h + 1) * ST1],
                            lhsT=wT_sb[:, ht, OLOC + h * P:OLOC + (h + 1) * P],
                            rhs=xt[:, ht, :],
                            start=ht == 0, stop=ht == NHT - 1,
                        )
                rope_block(pk, KT_sb, s0, cos, sin)

        # ---------------- phase 2: attention + interleaved o_proj ---------
        with (
            tc.tile_pool(name="p2e", bufs=3) as p2e,
            tc.tile_pool(name="p2acc", bufs=2) as p2acc,
            tc.tile_pool(name="p2rec", bufs=2) as p2rec,
            tc.tile_pool(name="p2a", bufs=4) as p2a,
            tc.tile_pool(name="p2st", bufs=3) as p2st,
            tc.tile_pool(name="p2pvs", bufs=2) as p2pvs,
            tc.tile_pool(name="p2sc", bufs=2, space="PSUM") as p2sc,
            tc.tile_pool(name="p2pv", bufs=1, space="PSUM") as p2pv,
            tc.tile_pool(name="p2po", bufs=2, space="PSUM") as p2po,
        ):
            A_tiles = [None] * 4            # A_tiles[t % 4] = attnT of tile t
            stg_cur = [None]
            pending = []                    # (tau, d) o_proj duos not yet issued
            ucount = [0]

            def oproj_duo(tau, d):
                # one of 8 o_proj duos for q-tile tau: sub = d // 2 rows,
                # htiles (2j, 2j+1) output columns
                sub, jp = divmod(d, 2)
                A = A_tiles[tau % 4]
                i = tau * (QT // P) + sub
                if jp == 0:
                    stg_cur[0] = p2st.tile([P, H], BF16, tag="stg",
                                           name=f"stg{tau}_{sub}")
                for k in range(2):
                    j = 2 * jp + k
                    po = p2po.tile([P, QT], F32, tag="po",
                                   name=f"po{tau}_{d}_{k}")
                    for oc in range(HPC):
                        nc.tensor.matmul(
                            po,
                            lhsT=A[:, oc * QT + sub * P:oc * QT + (sub + 1) * P],
                            rhs=woT_sb[:, oc, j * QT:(j + 1) * QT],
                            start=(oc == 0), stop=(oc == HPC - 1),
                        )
                    # PSUM drain: 1/3 ACT, 2/3 DVE (gpsimd cannot read
                    # PSUM; ACT is near its exp-paced ceiling late)
                    dst = stg_cur[0][:, j * QT:(j + 1) * QT]
                    ucount[0] += 1
                    if ucount[0] % 3 == 0:
                        nc.scalar.activation(
                            dst, po, mybir.ActivationFunctionType.Copy
                        )
                    else:
                        nc.vector.tensor_copy(dst, po)
                if jp == 1:
                    nc.sync.dma_start(out[i * P:(i + 1) * P, :], stg_cur[0])

            def make_epilogue(t, pvs, acc2):
                # fold/recip/norm for tile t, issued lazily inside tile
                # t+1's chunk loop so the boundary has no serial chain
                def epi():
                    rec = p2rec.tile([P, 2 * QT], F32, tag="rec",
                                     name=f"rec{t}")
                    for h in range(HPC):
                        fold = p2po.tile([P, QT], F32, tag="po",
                                         name=f"fold{t}_{h}")
                        for a in range(2):
                            nc.tensor.matmul(
                                fold,
                                lhsT=ones_sb,
                                rhs=acc2[a][:, h * QT:(h + 1) * QT],
                                start=(a == 0), stop=(a == 1),
                            )
                        nc.vector.reciprocal_approx_fast(
                            rec[:, h * QT:(h + 1) * QT], fold
                        )
                    A = p2a.tile([P, 2 * QT], BF16, tag="A", name=f"A{t}")
                    A_tiles[t % 4] = A
                    nc.vector.tensor_mul(A, pvs, rec)
                    pending.extend((t, d) for d in range(8))
                return epi

            epi_prev = None
            for t in range(NQT):
                q0 = t * QT
                nch = 4 * t + 4
                # host up to 2t pending o_proj duos in this tile's loop:
                # early tiles have little ACT/DVE slack, late tiles much.
                # The last tile holds 3 back so the post-loop flush pads
                # its own (unhosted) softmax epilogue chain.
                cap = 2 * t
                cap_in = cap if t < NQT - 1 else cap - 3
                issued = 0
                pv_ps = p2pv.tile([P, 2 * QT], F32, tag="pv", name=f"pv{t}")
                acc2 = [
                    p2acc.tile([P, 2 * QT], BF16, tag=f"acc{a}",
                               name=f"acc{a}_{t}")
                    for a in range(2)
                ]
                for c in range(nch):
                    first = c == 0
                    last = c == nch - 1
                    # diagonal chunk j covers only q >= 128j within the
                    # tile: narrow scores+exp to that range for j >= 2
                    # (not worth the extra exp instruction for j == 1);
                    # the full-width mask multiply zeroes the unwritten
                    # region of e, so PV/acc read zeros there.
                    j = c - 4 * t if c >= 4 * t else -1
                    qoff = 128 * j if j >= 2 else 0
                    sc = p2sc.tile([P, 2 * QT], F32, tag="sc")
                    for h in range(HPC):
                        nc.tensor.matmul(
                            sc[:, h * QT + qoff:(h + 1) * QT],
                            lhsT=KT_sb[:, h, c * P:(c + 1) * P],
                            rhs=QT_sb[:, h, q0 + qoff:q0 + QT],
                            start=True, stop=True,
                        )
                    e = p2e.tile([P, 2 * QT], BF16, tag="e")
                    if qoff:
                        for h in range(HPC):
                            nc.scalar.activation(
                                e[:, h * QT + qoff:(h + 1) * QT],
                                sc[:, h * QT + qoff:(h + 1) * QT],
                                mybir.ActivationFunctionType.Exp, scale=SCALE,
                            )
                    else:
                        nc.scalar.activation(
                            e, sc, mybir.ActivationFunctionType.Exp, scale=SCALE
                        )
                    if j >= 0:
                        nc.vector.tensor_mul(e, e, masks_sb[:, j, :])
                    # softmax denominator accumulation (k lives on
                    # partitions): two alternating bf16 accumulators so the
                    # serial add chain is 2x shorter; folded across
                    # partitions once per q-tile below.
                    acc = acc2[c % 2]
                    if c < 2:
                        nc.vector.tensor_copy(acc, e)
                    else:
                        nc.vector.tensor_add(acc, acc, e)
                    for h in range(HPC):
                        nc.tensor.matmul(
                            pv_ps[:, h * QT:(h + 1) * QT],
                            lhsT=V_sb[:, c, h * P:(h + 1) * P],
                            rhs=e[:, h * QT:(h + 1) * QT],
                            start=first, stop=last,
                        )
                    # previous tile's softmax epilogue becomes PE/DVE
                    # filler early in this tile's loop
                    if c == 1 and epi_prev is not None:
                        epi_prev()
                        epi_prev = None
                    # interleave pending o_proj duos so the PE has work
                    # while exp paces the chunk loop; floor-spread leaves
                    # the trailing chunks covered
                    if c >= 2 and issued < cap_in and pending:
                        k = min((cap_in - issued) // (nch - c),
                                cap_in - issued, len(pending))
                        for _ in range(k):
                            oproj_duo(*pending.pop(0))
                            issued += 1
                while issued < cap and pending:
                    oproj_duo(*pending.pop(0))
                    issued += 1

                # drain the unnormalized attn accumulator to SBUF so the
                # single PSUM pv buffer frees without waiting on
                # fold/recip/norm
                pvs = p2pvs.tile([P, 2 * QT], BF16, tag="pvs", name=f"pvs{t}")
                nc.vector.tensor_copy(pvs, pv_ps)
                epi = make_epilogue(t, pvs, acc2)
                if t < NQT - 1:
                    epi_prev = epi
                else:
                    epi()

            while pending:
                oproj_duo(*pending.pop(0))


def build_program():
    """Build + compile the Bass program (same program for all 8 cores)."""
    global _PROGRAM
    if _PROGRAM is not None:
        return _PROGRAM
    nc = bacc.Bacc(
        "TRN2", target_bir_lowering=False, debug=False, enable_asserts=False
    )
    with tile.TileContext(nc) as tc:
        _build_body(tc)
    nc.compile()
    _PROGRAM = nc
    return nc


def make_in_maps(hidden_states, w_qkv, w_o):
    import ml_dtypes

    x = np.asarray(hidden_states, dtype=np.float32).reshape(S, H)
    w = np.asarray(w_qkv, dtype=np.float32)
    wo = np.asarray(w_o, dtype=np.float32)

    xT = np.ascontiguousarray(x.T).astype(ml_dtypes.bfloat16)    # [2048, 4096]
    # [st, p, t, s]: per (st, p) the data is one contiguous 16 KiB run
    xTt = np.ascontiguousarray(
        xT.reshape(NHT, P, NQT, ST1).transpose(2, 1, 0, 3)
    )

    # rope tables, [128, 2, 8, 1024]: rows 0:64 and 64:128 both hold the
    # [64, S] table so the doubled layout lines up with [real; imag] dims;
    # per s-tile the 512-wide block is repeated twice so one [128, 1024]
    # DVE op covers both heads.
    e = np.arange(0, DH, 2, dtype=np.float32) / np.float32(DH)
    inv_freq = (1.0 / np.power(np.float32(10000.0), e)).astype(np.float32)
    t = np.arange(S, dtype=np.float32)
    freqs = np.outer(t, inv_freq).astype(np.float32)     # [S, 64]
    cosT = np.cos(freqs).T                               # [64, S]
    sinT = np.sin(freqs).T
    tabs = np.empty((P, 2, S), dtype=np.float32)
    tabs[0:64, 0] = cosT
    tabs[64:128, 0] = cosT
    tabs[0:64, 1] = sinT
    tabs[64:128, 1] = sinT
    rope = np.empty((P, 2, NQT, 2 * ST1), dtype=np.float32)
    for st in range(NQT):
        blk = tabs[:, :, st * ST1:(st + 1) * ST1]
        rope[:, :, st, 0:ST1] = blk
        rope[:, :, st, ST1:2 * ST1] = blk
    rope = rope.reshape(P, 2, NQT * 2 * ST1)

    # signed half-swap permutation: (J.T @ z)[d] = -z[64+d], [64+d] = +z[d]
    swapj = np.zeros((P, P), dtype=ml_dtypes.bfloat16)
    for d in range(64):
        swapj[64 + d, d] = -1.0
        swapj[d, 64 + d] = 1.0

    # diagonal-block masks [4, 128, 1024]: chunk at k0 = q0 + 128j keeps
    # (ki, qi) iff qi >= ki + 128j; tiled twice along q for the 2-head tile.
    ki = np.arange(P)[:, None]
    qi = np.arange(QT)[None, :]
    masks = np.empty((4, P, 2 * QT), dtype=ml_dtypes.bfloat16)
    for j in range(4):
        m = (qi >= ki + 128 * j).astype(ml_dtypes.bfloat16)
        masks[j] = np.concatenate([m, m], axis=1)

    in_maps = []
    for c in range(NCORES):
        r0 = c * OLOC
        w_loc = np.concatenate(
            [
                w[r0:r0 + OLOC],
                w[NH * DH + r0:NH * DH + r0 + OLOC],
                w[2 * NH * DH + r0:2 * NH * DH + r0 + OLOC],
            ],
            axis=0,
        )                                                # [768, 2048]
        wqkvT_c = w_loc.T.astype(ml_dtypes.bfloat16)     # [2048, 768]
        # [g, p, t, 256] with g = q/k/v: per (g, p) one contiguous 8 KiB run
        wqt_c = np.ascontiguousarray(
            wqkvT_c.reshape(NHT, P, 3, OLOC).transpose(2, 1, 0, 3)
        )
        woT_c = np.ascontiguousarray(
            wo[:, r0:r0 + OLOC].T
        ).astype(ml_dtypes.bfloat16)                     # [256, 2048]
        in_maps.append(
            {
                "xTt": xTt,
                "wqt": wqt_c,
                "woT": woT_c,
                "rope": rope,
                "swapj": swapj,
                "onesin": np.ones((P, P), dtype=ml_dtypes.bfloat16),
                "masks": masks,
            }
        )
    return in_maps


def run_cores(in_maps, trace=False, **kwargs):
    nc = build_program()
    return run_bass_kernel_spmd(
        nc, in_maps, list(range(NCORES)), trace=trace, **kwargs
    )


def kernel(hidden_states, w_qkv, w_o):
    in_maps = make_in_maps(hidden_states, w_qkv, w_o)
    res = run_cores(in_maps)
    acc = res.results[0]["out"].astype(np.float32)
    for c in range(1, NCORES):
        acc = acc + res.results[c]["out"].astype(np.float32)
    return acc.reshape(1, S, H)


# revision 49
# speedup vs baseline: 1.1992x; 1.0008x over previous
"""Trainium2 Bass kernel for a 16-head causal self-attention block.

Reference computation (B=1, S=4096, H=2048, 16 heads x 128 dim, fp32):
    qkv = x @ w_qkv.T            # [S, 6144]
    q, k = rope(q), rope(k)      # half-split rope
    attn = causal_softmax(q k^T / sqrt(128)) @ v
    out  = attn @ w_o.T          # [S, 2048]

Sharding: tensor-parallel over heads.  Each of the 8 cores owns 2 heads:
it computes its slice of the QKV projection (768 rows), attention for its
2 heads, and a partial o_proj ([S, 2048], bf16); the host sums the 8
partials.

Dataflow per core (matmul operands bf16, accumulation/softmax fp32):
  phase 1 (QKV+rope), per s-tile of 512: one batched x^T DMA group per
    tile, then three matmul sweeps (V, Q, K) rotating over three PSUM
    slots so each sweep's epilogue drains while the next computes.  RoPE
    is fused into the Q/K epilogues; rotate_half is a signed-permutation
    matmul on the PE; cos/sin tables are pre-doubled so one [128,1024]
    DVE op covers both heads.
  phase 2 (attention), per q-tile of 512: loop causal k-chunks of 128:
    scoresT[k,q] -> ACT exp (both heads in one [128,1024] instr) ->
    diagonal-mask multiply -> PV matmul, with bf16 softmax-denominator
    accumulators (two alternating, halves the serial add chain); per
    q-tile a ones-matmul folds the partition sums, fast-reciprocal +
    one multiply normalizes attn^T.  o_proj for q-tile t-1 is
    interleaved into tile t's chunk loop (PE absorbs the exp-paced
    slack); its PSUM drains go to the otherwise-idle GpSimd engine.
"""

import numpy as np

import concourse.bass as bass
import concourse.mybir as mybir
import concourse.tile as tile
from concourse import bacc
from concourse.bass_utils import run_bass_kernel_spmd

F32 = mybir.dt.float32
BF16 = mybir.dt.bfloat16

S = 4096
H = 2048
DH = 128
NH = 16
NCORES = 8
HPC = NH // NCORES          # 2 heads per core
OLOC = HPC * DH             # 256 local o-channels per q/k/v group
P = 128
ST1 = 512                   # phase-1 s-tile width
NHT = H // P                # 16 h-chunks
QT = 512                    # phase-2 q-tile width
NQT = S // QT               # 8 q-tiles
NKC = S // P                # 32 k-chunks
SCALE = 1.0 / float(np.sqrt(np.float32(DH)))

_PROGRAM = None


def _build_body(tc):
    nc = tc.nc

    # x and w_qkv are host-retiled so each partition's chunk is one long
    # contiguous run (big DMA descriptors: ~25ns fixed cost per descriptor)
    xTt = nc.dram_tensor("xTt", [NQT, P, NHT, ST1], BF16, kind="ExternalInput").ap()
    wqt = nc.dram_tensor("wqt", [3, P, NHT, OLOC], BF16, kind="ExternalInput").ap()
    woT = nc.dram_tensor("woT", [OLOC, H], BF16, kind="ExternalInput").ap()
    rope = nc.dram_tensor("rope", [P, 2, NQT * 2 * ST1], F32, kind="ExternalInput").ap()
    swapj = nc.dram_tensor("swapj", [P, P], BF16, kind="ExternalInput").ap()
    onesin = nc.dram_tensor("onesin", [P, P], BF16, kind="ExternalInput").ap()
    masks = nc.dram_tensor("masks", [4, P, 2 * QT], BF16, kind="ExternalInput").ap()
    out = nc.dram_tensor("out", [S, H], BF16, kind="ExternalOutput").ap()

    woT_v = woT.rearrange("(t p) h -> p t h", p=P)      # [128, 2, 2048]

    with tc.tile_pool(name="resident", bufs=1) as resident:
        # d-major Q^T/K^T: [128 d, head, s]; s-major V: [128 s, k-chunk, 256]
        QT_sb = resident.tile([P, HPC, S], BF16)
        KT_sb = resident.tile([P, HPC, S], BF16)
        V_sb = resident.tile([P, NKC, OLOC], BF16)
        wT_sb = resident.tile([P, NHT, 3 * OLOC], BF16)
        woT_sb = resident.tile([P, HPC, H], BF16)
        masks_sb = resident.tile([P, 4, 2 * QT], BF16)
        ones_sb = resident.tile([P, P], BF16)
        J_sb = resident.tile([P, P], BF16)

        # All phase-1 inputs go on the sync queue in consumption order so
        # the first-needed transfers get the full DMA bandwidth instead of
        # fair-sharing it with later ones (issue spacing staggers them).
        nc.gpsimd.dma_start(J_sb, swapj)

        # ---------------- phase 1: QKV projection + rope ----------------
        with (
            tc.tile_pool(name="p1x", bufs=3) as p1x,
            tc.tile_pool(name="p1tab", bufs=3) as p1tab,
            tc.tile_pool(name="p1t1", bufs=2) as p1t1,
            tc.tile_pool(name="p1t2", bufs=2) as p1t2,
            tc.tile_pool(name="p1ps", bufs=3, space="PSUM") as p1ps,
            tc.tile_pool(name="p1rot", bufs=1, space="PSUM") as p1rot,
        ):
            def rope_block(blk, dst, s0, cos, sin):
                # blk: [128, 1024] PSUM (both heads); dst: QT_sb or KT_sb
                t1 = p1t1.tile([P, 2 * ST1], F32, tag="t1")
                t2 = p1t2.tile([P, 2 * ST1], BF16, tag="t2")
                nc.vector.tensor_mul(t1, blk, cos)
                nc.vector.tensor_mul(t2, blk, sin)
                rot = p1rot.tile([P, 2 * ST1], F32, tag="rot")
                nc.tensor.matmul(rot[:, 0:ST1], lhsT=J_sb, rhs=t2[:, 0:ST1],
                                 start=True, stop=True)
                nc.tensor.matmul(rot[:, ST1:2 * ST1], lhsT=J_sb,
                                 rhs=t2[:, ST1:2 * ST1], start=True, stop=True)
                for h in range(HPC):
                    nc.vector.tensor_add(
                        dst[:, h, s0:s0 + ST1],
                        t1[:, h * ST1:(h + 1) * ST1],
                        rot[:, h * ST1:(h + 1) * ST1],
                    )

            for st in range(S // ST1):
                s0 = st * ST1
                xt = p1x.tile([P, NHT, ST1], BF16, tag="xt", name=f"xt{st}")
                if st == 0:
                    # V columns gate the first sweep; Q/K columns follow
                    # behind the tile-0 x loads (g-major host layout keeps
                    # every one of these a large-descriptor transfer)
                    nc.sync.dma_start(
                        wT_sb[:, :, 2 * OLOC:3 * OLOC], wqt[2]
                    )
                for g in range(4):
                    nc.sync.dma_start(
                        xt[:, 4 * g:4 * (g + 1), :],
                        xTt[st, :, 4 * g:4 * (g + 1), :],
                    )
                if st == 0:
                    nc.sync.dma_start(wT_sb[:, :, 0:OLOC], wqt[0])
                    nc.sync.dma_start(wT_sb[:, :, OLOC:2 * OLOC], wqt[1])
                tab = p1tab.tile([P, 2, 2 * ST1], F32, tag="tab")
                nc.sync.dma_start(
                    tab, rope[:, :, st * 2 * ST1:(st + 1) * 2 * ST1]
                )
                if st == 2:
                    # phase-2-only tensors: issued on the scalar queue here,
                    # behind two tiles of V-drain copies, so their traffic
                    # cannot crowd out the startup-critical loads above
                    nc.scalar.dma_start(ones_sb, onesin)
                    nc.scalar.dma_start(masks_sb, masks.rearrange("j p q -> p j q"))
                    nc.scalar.dma_start(woT_sb, woT_v)
                cos = tab[:, 0, :]
                sin = tab[:, 1, :]

                # --- V sweep ---
                # [128, 1024]: s-subs 0,1 share zero-region/bank 0; 2,3 share 1
                pv = p1ps.tile([P, 2 * ST1], F32, tag="qkv", name=f"pv{st}")
                for ht in range(NHT):
                    for sub in range(ST1 // P):
                        nc.tensor.matmul(
                            pv[:, sub * OLOC:(sub + 1) * OLOC],
                            lhsT=xt[:, ht, sub * P:(sub + 1) * P],
                            rhs=wT_sb[:, ht, 2 * OLOC:3 * OLOC],
                            start=(ht == 0) and sub % 2 == 0,
                            stop=(ht == NHT - 1) and sub % 2 == 1,
                        )
                for sub in range(ST1 // P):
                    nc.scalar.activation(
                        V_sb[:, st * (ST1 // P) + sub, :],
                        pv[:, sub * OLOC:(sub + 1) * OLOC],
                        mybir.ActivationFunctionType.Copy,
                    )

                # --- Q sweep + rope ---
                pq = p1ps.tile([P, 2 * ST1], F32, tag="qkv", name=f"pq{st}")
                for ht in range(NHT):
                    for h in range(HPC):
                        nc.tensor.matmul(
                            pq[:, h * ST1:(h + 1) * ST1],
                            lhsT=wT_sb[:, ht, h * P:(h + 1) * P],
                            rhs=xt[:, ht, :],
                            start=ht == 0, stop=ht == NHT - 1,
                        )
                rope_block(pq, QT_sb, s0, cos, sin)

                # --- K sweep + rope ---
                pk = p1ps.tile([P, 2 * ST1], F32, tag="qkv", name=f"pk{st}")
                for ht in range(NHT):
                    for h in range(HPC):
                        nc.tensor.matmul(
                            pk[:, h * ST1:(h + 1) * ST1],
                            lhsT=wT_sb[:, ht, OLOC + h * P:OLOC + (h + 1) * P],
                            rhs=xt[:, ht, :],
                            start=ht == 0, stop=ht == NHT - 1,
                        )
                rope_block(pk, KT_sb, s0, cos, sin)

        # ---------------- phase 2: attention + interleaved o_proj ---------
        with (
            tc.tile_pool(name="p2e", bufs=4) as p2e,
            tc.tile_pool(name="p2acc", bufs=2) as p2acc,
            tc.tile_pool(name="p2rec", bufs=2) as p2rec,
            tc.tile_pool(name="p2a", bufs=4) as p2a,
            tc.tile_pool(name="p2st", bufs=4) as p2st,
            tc.tile_pool(name="p2pvs", bufs=2) as p2pvs,
            tc.tile_pool(name="p2sc", bufs=2, space="PSUM") as p2sc,
            tc.tile_pool(name="p2pv", bufs=1, space="PSUM") as p2pv,
            tc.tile_pool(name="p2po", bufs=2, space="PSUM") as p2po,
        ):
            A_tiles = [None] * 4            # A_tiles[t % 4] = attnT of tile t
            stg_cur = [None]
            pending = []                    # (tau, d) o_proj duos not yet issued
            ucount = [0]

            def oproj_duo(tau, d):
                # one of 8 o_proj duos for q-tile tau: sub = d // 2 rows,
                # htiles (2j, 2j+1) output columns
                sub, jp = divmod(d, 2)
                A = A_tiles[tau % 4]
                i = tau * (QT // P) + sub
                if jp == 0:
                    stg_cur[0] = p2st.tile([P, H], BF16, tag="stg",
                                           name=f"stg{tau}_{sub}")
                for k in range(2):
                    j = 2 * jp + k
                    po = p2po.tile([P, QT], F32, tag="po",
                                   name=f"po{tau}_{d}_{k}")
                    for oc in range(HPC):
                        nc.tensor.matmul(
                            po,
                            lhsT=A[:, oc * QT + sub * P:oc * QT + (sub + 1) * P],
                            rhs=woT_sb[:, oc, j * QT:(j + 1) * QT],
                            start=(oc == 0), stop=(oc == HPC - 1),
                        )
                    # PSUM drain: 1/3 ACT, 2/3 DVE (gpsimd cannot read
                    # PSUM; ACT is near its exp-paced ceiling late)
                    dst = stg_cur[0][:, j * QT:(j + 1) * QT]
                    ucount[0] += 1
                    if ucount[0] % 3 == 0:
                        nc.scalar.activation(
                            dst, po, mybir.ActivationFunctionType.Copy
                        )
                    else:
                        nc.vector.tensor_copy(dst, po)
                if jp == 1:
                    nc.sync.dma_start(out[i * P:(i + 1) * P, :], stg_cur[0])

            def make_epilogue(t, pvs, acc2):
                # fold/recip/norm for tile t, issued lazily inside tile
                # t+1's chunk loop so the boundary has no serial chain
                def epi():
                    rec = p2rec.tile([P, 2 * QT], F32, tag="rec",
                                     name=f"rec{t}")
                    for h in range(HPC):
                        fold = p2po.tile([P, QT], F32, tag="po",
                                         name=f"fold{t}_{h}")
                        for a in range(2):
                            nc.tensor.matmul(
                                fold,
                                lhsT=ones_sb,
                                rhs=acc2[a][:, h * QT:(h + 1) * QT],
                                start=(a == 0), stop=(a == 1),
                            )
                        nc.vector.reciprocal_approx_fast(
                            rec[:, h * QT:(h + 1) * QT], fold
                        )
                    A = p2a.tile([P, 2 * QT], BF16, tag="A", name=f"A{t}")
                    A_tiles[t % 4] = A
                    nc.vector.tensor_mul(A, pvs, rec)
                    pending.extend((t, d) for d in range(8))
                return epi

            epi_prev = None
            for t in range(NQT):
                q0 = t * QT
                nch = 4 * t + 4
                # host up to 2t pending o_proj duos in this tile's loop:
                # early tiles have little ACT/DVE slack, late tiles much.
                # The last tile holds 3 back so the post-loop flush pads
                # its own (unhosted) softmax epilogue chain.
                cap = 2 * t
                cap_in = cap if t < NQT - 1 else cap - 3
                issued = 0
                pv_ps = p2pv.tile([P, 2 * QT], F32, tag="pv", name=f"pv{t}")
                acc2 = [
                    p2acc.tile([P, 2 * QT], BF16, tag=f"acc{a}",
                               name=f"acc{a}_{t}")
                    for a in range(2)
                ]
                for c in range(nch):
                    first = c == 0
                    last = c == nch - 1
                    # diagonal chunk j covers only q >= 128j within the
                    # tile: narrow scores+exp to that range for j >= 2
                    # (not worth the extra exp instruction for j == 1);
                    # the full-width mask multiply zeroes the unwritten
                    # region of e, so PV/acc read zeros there.
                    j = c - 4 * t if c >= 4 * t else -1
                    qoff = 128 * j if j >= 2 else 0
                    sc = p2sc.tile([P, 2 * QT], F32, tag="sc")
                    for h in range(HPC):
                        nc.tensor.matmul(
                            sc[:, h * QT + qoff:(h + 1) * QT],
                            lhsT=KT_sb[:, h, c * P:(c + 1) * P],
                            rhs=QT_sb[:, h, q0 + qoff:q0 + QT],
                            start=True, stop=True,
                        )
                    e = p2e.tile([P, 2 * QT], BF16, tag="e")
                    if qoff:
                        for h in range(HPC):
                            nc.scalar.activation(
                                e[:, h * QT + qoff:(h + 1) * QT],
                                sc[:, h * QT + qoff:(h + 1) * QT],
                                mybir.ActivationFunctionType.Exp, scale=SCALE,
                            )
                    else:
                        nc.scalar.activation(
                            e, sc, mybir.ActivationFunctionType.Exp, scale=SCALE
                        )
                    if j >= 0:
                        nc.vector.tensor_mul(e, e, masks_sb[:, j, :])
                    # softmax denominator accumulation (k lives on
                    # partitions): two alternating bf16 accumulators so the
                    # serial add chain is 2x shorter; folded across
                    # partitions once per q-tile below.
                    acc = acc2[c % 2]
                    if c < 2:
                        nc.vector.tensor_copy(acc, e)
                    else:
                        nc.vector.tensor_add(acc, acc, e)
                    for h in range(HPC):
                        nc.tensor.matmul(
                            pv_ps[:, h * QT:(h + 1) * QT],
                            lhsT=V_sb[:, c, h * P:(h + 1) * P],
                            rhs=e[:, h * QT:(h + 1) * QT],
                            start=first, stop=last,
                        )
                    # previous tile's softmax epilogue becomes PE/DVE
                    # filler early in this tile's loop
                    if c == 1 and epi_prev is not None:
                        epi_prev()
                        epi_prev = None
                    # interleave pending o_proj duos so the PE has work
                    # while exp paces the chunk loop; floor-spread leaves
                    # the trailing chunks covered
                    if c >= 2 and issued < cap_in and pending:
                        k = min((cap_in - issued) // (nch - c),
                                cap_in - issued, len(pending))
                        for _ in range(k):
                            oproj_duo(*pending.pop(0))
                            issued += 1
                while issued < cap and pending:
                    oproj_duo(*pending.pop(0))
                    issued += 1

                # drain the unnormalized attn accumulator to SBUF so the
                # single PSUM pv buffer frees without waiting on
                # fold/recip/norm
                pvs = p2pvs.tile([P, 2 * QT], BF16, tag="pvs", name=f"pvs{t}")
                nc.vector.tensor_copy(pvs, pv_ps)
                epi = make_epilogue(t, pvs, acc2)
                if t < NQT - 1:
                    epi_prev = epi
                else:
                    epi()

            while pending:
                oproj_duo(*pending.pop(0))


def build_program():
    """Build + compile the Bass program (same program for all 8 cores)."""
    global _PROGRAM
    if _PROGRAM is not None:
        return _PROGRAM
    nc = bacc.Bacc(
        "TRN2", target_bir_lowering=False, debug=False, enable_asserts=False
    )
    with tile.TileContext(nc) as tc:
        _build_body(tc)
    nc.compile()
    _PROGRAM = nc
    return nc


def make_in_maps(hidden_states, w_qkv, w_o):
    import ml_dtypes

    x = np.asarray(hidden_states, dtype=np.float32).reshape(S, H)
    w = np.asarray(w_qkv, dtype=np.float32)
    wo = np.asarray(w_o, dtype=np.float32)

    xT = np.ascontiguousarray(x.T).astype(ml_dtypes.bfloat16)    # [2048, 4096]
    # [st, p, t, s]: per (st, p) the data is one contiguous 16 KiB run
    xTt = np.ascontiguousarray(
        xT.reshape(NHT, P, NQT, ST1).transpose(2, 1, 0, 3)
    )

    # rope tables, [128, 2, 8, 1024]: rows 0:64 and 64:128 both hold the
    # [64, S] table so the doubled layout lines up with [real; imag] dims;
    # per s-tile the 512-wide block is repeated twice so one [128, 1024]
    # DVE op covers both heads.
    e = np.arange(0, DH, 2, dtype=np.float32) / np.float32(DH)
    inv_freq = (1.0 / np.power(np.float32(10000.0), e)).astype(np.float32)
    t = np.arange(S, dtype=np.float32)
    freqs = np.outer(t, inv_freq).astype(np.float32)     # [S, 64]
    cosT = np.cos(freqs).T                               # [64, S]
    sinT = np.sin(freqs).T
    tabs = np.empty((P, 2, S), dtype=np.float32)
    tabs[0:64, 0] = cosT
    tabs[64:128, 0] = cosT
    tabs[0:64, 1] = sinT
    tabs[64:128, 1] = sinT
    rope = np.empty((P, 2, NQT, 2 * ST1), dtype=np.float32)
    for st in range(NQT):
        blk = tabs[:, :, st * ST1:(st + 1) * ST1]
        rope[:, :, st, 0:ST1] = blk
        rope[:, :, st, ST1:2 * ST1] = blk
    rope = rope.reshape(P, 2, NQT * 2 * ST1)

    # signed half-swap permutation: (J.T @ z)[d] = -z[64+d], [64+d] = +z[d]
    swapj = np.zeros((P, P), dtype=ml_dtypes.bfloat16)
    for d in range(64):
        swapj[64 + d, d] = -1.0
        swapj[d, 64 + d] = 1.0

    # diagonal-block masks [4, 128, 1024]: chunk at k0 = q0 + 128j keeps
    # (ki, qi) iff qi >= ki + 128j; tiled twice along q for the 2-head tile.
    ki = np.arange(P)[:, None]
    qi = np.arange(QT)[None, :]
    masks = np.empty((4, P, 2 * QT), dtype=ml_dtypes.bfloat16)
    for j in range(4):
        m = (qi >= ki + 128 * j).astype(ml_dtypes.bfloat16)
        masks[j] = np.concatenate([m, m], axis=1)

    in_maps = []
    for c in range(NCORES):
        r0 = c * OLOC
        w_loc = np.concatenate(
            [
                w[r0:r0 + OLOC],
                w[NH * DH + r0:NH * DH + r0 + OLOC],
                w[2 * NH * DH + r0:2 * NH * DH + r0 + OLOC],
            ],
            axis=0,
        )                                                # [768, 2048]
        wqkvT_c = w_loc.T.astype(ml_dtypes.bfloat16)     # [2048, 768]
        # [g, p, t, 256] with g = q/k/v: per (g, p) one contiguous 8 KiB run
        wqt_c = np.ascontiguousarray(
            wqkvT_c.reshape(NHT, P, 3, OLOC).transpose(2, 1, 0, 3)
        )
        woT_c = np.ascontiguousarray(
            wo[:, r0:r0 + OLOC].T
        ).astype(ml_dtypes.bfloat16)                     # [256, 2048]
        in_maps.append(
            {
                "xTt": xTt,
                "wqt": wqt_c,
                "woT": woT_c,
                "rope": rope,
                "swapj": swapj,
                "onesin": np.ones((P, P), dtype=ml_dtypes.bfloat16),
                "masks": masks,
            }
        )
    return in_maps


def run_cores(in_maps, trace=False, **kwargs):
    nc = build_program()
    return run_bass_kernel_spmd(
        nc, in_maps, list(range(NCORES)), trace=trace, **kwargs
    )


def kernel(hidden_states, w_qkv, w_o):
    in_maps = make_in_maps(hidden_states, w_qkv, w_o)
    res = run_cores(in_maps)
    acc = res.results[0]["out"].astype(np.float32)
    for c in range(1, NCORES):
        acc = acc + res.results[c]["out"].astype(np.float32)
    return acc.reshape(1, S, H)
